# revision 16
# baseline (speedup 1.0000x reference)
"""Trainium2 Bass kernel for nn_BasicTransformerLayer (dense transformer layer).

Strategy v2:
- Data-parallel over batch: B=8, one batch element per NeuronCore, no
  collectives.
- Activations transposed [features, tokens]; residual stream in fp32r,
  everything on matmul paths in float16 (weights pre-tiled on host into
  contiguous per-output-tile blocks for max DMA efficiency).
- Softmax: exp(scores) on scalar engine (f16), multiplicative exp(bias)
  (host-precomputed, f16) on vector; the kv-sum (softmax denominator) is
  folded into the P@V matmul via V tiles laid out [64 ones | 64 values]
  per head: psum rows 0:64 = sums, 64:128 = P@V.
- DMA spread across sync/scalar/gpsimd rings; output written per-psum-bank
  with rotating buffers to avoid a serialized tail.
"""
import sys

sys.path.insert(0, '/opt/trn_rl_repo')

import numpy as np

E, C, H, D, FF = 768, 512, 12, 64, 3072
B, S, L = 8, 1024, 256
EPS = 1e-5
NCORES = 8
QCH = 512                  # q-chunk (matmul moving free dim)
NQ = S // QCH              # 2
JE = E // 128              # 6 feature tiles
JC = C // 128              # 4 cross-feature tiles
JF = FF // 128             # 24 ffn tiles
KVS = S // 128             # 8 self kv tiles
KVC = L // 128             # 2 cross kv tiles

_BUILT = {}
TRACE = False
LAST = {}
PHASES = []


def _build(flags):
    import concourse.bacc as bacc
    import concourse.mybir as mybir
    import concourse.tile as tile
    from concourse.tile import add_dep_helper
    from concourse.bass import AP as BassAP

    FR = mybir.dt.float32r
    F32 = mybir.dt.float32
    F16 = mybir.dt.float16
    AF = mybir.ActivationFunctionType
    OP = mybir.AluOpType

    nc = bacc.Bacc("TRN2", target_bir_lowering=False, debug=False,
                   enable_asserts=True, num_devices=NCORES)

    def din(name, shape, dt=F16):
        return nc.dram_tensor(name, shape, dt, kind="ExternalInput").ap()

    xT_d = din("xT", [E, S], F32)
    ctxT_d = din("ctxT", [C, L])
    # pre-tiled weights: [ofn, 128, jin*128] f16 (contiguous per of)
    w_d = {
        'wq_c': din("wq_c", [JE, 128, JE * 128]),
        'wk_c': din("wk_c", [JE, 128, JC * 128]),
        'wo_c': din("wo_c", [JE, 128, JE * 128]),
        'wq_s': din("wq_s", [JE, 128, JE * 128]),
        'wk_s': din("wk_s", [JE, 128, JE * 128]),
        'wo_s': din("wo_s", [JE, 128, JE * 128]),
        'w1': din("w1", [JF, 128, JE * 128]),
        'w2': din("w2", [JF, 128, JE * 128]),
        # V weights: [jin, 128, E] (row blocks of original)
        'wv_c': din("wv_c", [JC, 128, E]),
        'wv_s': din("wv_s", [JE, 128, E]),
    }
    expb_c_d = din("expb_c", [H, L, S])
    expb_s_d = din("expb_s", [H, S, S])
    VIDX = {}
    _off = 0
    for nm, ln in [('cn_g', JE), ('cn_b', JE), ('sn_g', JE), ('sn_b', JE),
                   ('fn_g', JE), ('fn_b', JE), ('bq_c', JE), ('bk_c', JE),
                   ('bo_c', JE), ('bq_s', JE), ('bk_s', JE), ('bo_s', JE),
                   ('b1', JF), ('b2', JE)]:
        VIDX[nm] = _off
        _off += ln
    NV = _off
    vecs_d = din("vecs", [128, NV], F32)
    yT_d = nc.dram_tensor("yT", [E, S], F32, kind="ExternalOutput").ap()

    with tile.TileContext(nc) as tc:
        with tc.tile_pool(name="const", bufs=1) as cpool, \
             tc.tile_pool(name="acts", bufs=1) as acts, \
             tc.tile_pool(name="wst", bufs=8) as wst, \
             tc.tile_pool(name="tr", bufs=2) as tr, \
             tc.tile_pool(name="pe", bufs=4) as pepool, \
             tc.tile_pool(name="eb", bufs=4) as ebpool, \
             tc.tile_pool(name="ps", bufs=1, space="PSUM") as ps:

            def T(pool, shape, dtype, tag, bufs=1):
                return pool.tile(shape, dtype, tag=tag, name=tag, bufs=bufs)

            ones = T(cpool, [128, 128], FR, "ones")
            ones16 = T(cpool, [128, 128], F16, "ones16")
            ones_f = T(cpool, [128, 128], F32, "ones_f")
            epsc = T(cpool, [128, 1], F32, "epsc")
            nc.vector.memset(epsc[:], EPS)
            nc.vector.memset(ones_f[:], 1.0)
            nc.vector.tensor_copy(ones[:], ones_f[:])
            nc.vector.tensor_copy(ones16[:], ones_f[:])
            vecs = T(cpool, [128, NV], F32, "vecs")
            nc.sync.dma_start(vecs[:], vecs_d[:])

            def vap(nm, j):
                return vecs[:, VIDX[nm] + j:VIDX[nm] + j + 1]

            # persistent activation tiles
            rA = [T(acts, [128, S], FR, f"rA{j}") for j in range(JE)]
            rB = [T(acts, [128, S], FR, f"rB{j}") for j in range(JE)]
            lnT = [T(acts, [128, S], F16, f"ln{j}") for j in range(JE)]
            KT = [T(acts, [128, S], F16, f"KT{j}") for j in range(JE)]
            # V tiles: per head 128 cols = [64 ones | 64 values]
            Vx = [T(acts, [128, H * 128], F16, f"V{t}") for t in range(KVS)]
            QT = [T(acts, [128, QCH], F16, f"QT{j}") for j in range(JE)]
            AT = [T(acts, [128, QCH], F16, f"AT{j}") for j in range(JE)]
            ctxT = [T(acts, [128, L], F16, f"cx{j}") for j in range(JC)]

            def vx_strided(t, head0, nh, ones_cols):
                vp = Vx[t][:]
                pstride = vp.ap[0][0]
                off = vp.offset + head0 * 128 + (0 if ones_cols else 64)
                return BassAP(vp.tensor, off,
                              [[pstride, 128], [128, nh], [1, 64]])

            for t in range(KVS):
                nc.vector.memset(vx_strided(t, 0, H, True), 1.0)

            for j in range(JC):
                nc.sync.dma_start(ctxT[j][:], ctxT_d[j * 128:(j + 1) * 128, :])
            # xT in column halves so ln1 qc0 can start earlier
            for half in range(2):
                cs_ = slice(half * QCH, (half + 1) * QCH)
                for j in range(JE):
                    nc.scalar.dma_start(
                        rA[j][:, cs_],
                        xT_d[j * 128:(j + 1) * 128, cs_].bitcast(FR))

            # PSUM: 3x double-bank "scp" tiles (scores/general) + 2 single
            # "pv" banks = 8 banks total.
            def psum_tile(tag, n=QCH):
                return ps.tile([128, n], F32, tag=tag, name=tag)

            def scp_tile(k):
                return psum_tile(f"scp{k % 3}", 2 * QCH)

            _rot = {'i': 0, 'cur': None}

            def rot_ps(n=QCH):
                i = _rot['i']
                _rot['i'] += 1
                if i % 2 == 0:
                    _rot['cur'] = scp_tile(i // 2)
                return _rot['cur'][:, (i % 2) * QCH:(i % 2) * QCH + n]

            # ---------------- layer norm (transposed layout) ----------------
            def ln_phase(src, dst, gname, bname, affine, only_qc=None):
                inv = 1.0 / float(E)
                for qc in range(NQ):
                    if only_qc is not None and qc != only_qc:
                        continue
                    qs = slice(qc * QCH, (qc + 1) * QCH)
                    sqs = []
                    for j in range(JE):
                        sq = T(tr, [128, QCH], F16, "sq", bufs=6)
                        nc.scalar.activation(sq[:], src[j][:, qs], AF.Square)
                        sqs.append(sq)
                    s12 = scp_tile(0)
                    s1 = s12[:, 0:QCH]
                    s2 = s12[:, QCH:2 * QCH]
                    for j in range(JE):
                        nc.tensor.matmul(s1, ones[:, 0:128], src[j][:, qs],
                                         start=(j == 0), stop=(j == JE - 1))
                    for j in range(JE):
                        nc.tensor.matmul(s2, ones16[:, 0:128], sqs[j][:],
                                         start=(j == 0), stop=(j == JE - 1))
                    t1 = T(tr, [128, QCH], F32, "t1m", bufs=1)
                    nc.scalar.activation(t1[:], s1[:], AF.Square, scale=inv)
                    var = T(tr, [128, QCH], F32, "var", bufs=1)
                    nc.vector.scalar_tensor_tensor(var[:], s2[:], inv, t1[:],
                                                   op0=OP.mult, op1=OP.subtract)
                    nc.scalar.activation(var[:], var[:], AF.Sqrt,
                                         bias=epsc[:, 0:1])
                    rstd = T(tr, [128, QCH], F32, "rstd", bufs=1)
                    nc.vector.reciprocal_approx_fast(rstd[:], var[:])
                    m1r = T(tr, [128, QCH], F32, "m1r", bufs=1)
                    nc.vector.scalar_tensor_tensor(m1r[:], s1[:], inv, rstd[:],
                                                   op0=OP.mult, op1=OP.mult)
                    for j in range(JE):
                        tmp = T(tr, [128, QCH], F32, "lntmp", bufs=2)
                        nc.vector.tensor_tensor(tmp[:], src[j][:, qs], rstd[:],
                                                op=OP.mult)
                        if affine:
                            tmp2 = T(tr, [128, QCH], F32, "lntmp2", bufs=2)
                            nc.vector.tensor_tensor(tmp2[:], tmp[:], m1r[:],
                                                    op=OP.subtract)
                            nc.vector.tensor_scalar(dst[j][:, qs], tmp2[:],
                                                    vap(gname, j), vap(bname, j),
                                                    op0=OP.mult, op1=OP.add)
                        else:
                            nc.vector.tensor_tensor(dst[j][:, qs], tmp[:],
                                                    m1r[:], op=OP.subtract)

            # -------- projection from pre-tiled weights --------
            def wload(wd, of, jin, ring=None):
                wt = T(wst, [128, JE * 128], F16, "wg", bufs=6)
                (ring or nc.sync).dma_start(wt[:, 0:jin * 128], wd[of])
                return wt

            _cpn = {'i': 0}

            def wchain(wt, jin, src_getter, out_ap, n, bias_ap, pt=None):
                if pt is None:
                    pt = rot_ps(n)
                for j in range(jin):
                    nc.tensor.matmul(pt, wt[:, j * 128:(j + 1) * 128],
                                     src_getter(j),
                                     start=(j == 0), stop=(j == jin - 1))
                if bias_ap is not None:
                    nc.vector.tensor_scalar(out_ap, pt, bias_ap, None,
                                            op0=OP.add)
                elif _cpn['i'] % 2 == 0:
                    _cpn['i'] += 1
                    nc.scalar.copy(out_ap, pt)
                else:
                    _cpn['i'] += 1
                    nc.vector.tensor_copy(out_ap, pt)

            # ---------------- K/V projection emission ----------------
            def emit_kv(prefix, kv_src, wk, wv, jin_kv, kv_len):
                nkv = kv_len // 128

                _kps = {'i': 0}

                def emit_k(of):
                    wt = wload(wk, of, jin_kv)
                    for ks in range(0, kv_len, QCH):
                        n = min(QCH, kv_len - ks)
                        kp = psum_tile(f"pv{_kps['i'] % 2}")
                        _kps['i'] += 1
                        wchain(wt, jin_kv,
                               lambda j: kv_src[j][:, ks:ks + n],
                               KT[of][:, ks:ks + n], n,
                               vap(f'bk_{prefix}', of)
                               if flags[f'bk_{prefix}'] else None,
                               pt=kp[:, 0:n])

                def emit_vgroup(os_, tg):
                    n = min(QCH, E - os_)
                    tcnt = min(4, nkv - tg)
                    vts = [scp_tile(1), scp_tile(2)]
                    vps = [vts[i // 2][:, (i % 2) * QCH:(i % 2) * QCH + n]
                           for i in range(tcnt)]
                    for j in range(jin_kv):
                        wt = T(wst, [128, QCH], F16, "wv", bufs=3)
                        nc.sync.dma_start(wt[:, 0:n], wv[j, :, os_:os_ + n])
                        for i in range(tcnt):
                            nc.tensor.matmul(
                                vps[i][:, 0:n],
                                kv_src[j][:, (tg + i) * 128:(tg + i + 1) * 128],
                                wt[:, 0:n], start=(j == 0),
                                stop=(j == jin_kv - 1))
                    for i in range(tcnt):
                        dst = vx_strided(tg + i, os_ // 64, n // 64, False)
                        src = vps[i][:, 0:n].rearrange("p (h d) -> p h d", d=64)
                        nc.scalar.copy(dst, src)

                vgroups = [(os_, tg) for os_ in range(0, E, QCH)
                           for tg in range(0, nkv, 4)]
                for i in range(max(JE, len(vgroups))):
                    if i < JE:
                        emit_k(i)
                    if i < len(vgroups):
                        emit_vgroup(*vgroups[i])

            # ---------------- attention (shared cross/self) ----------------
            def attention(prefix, lnt, kv_src, expb_d, res_in,
                          res_out, wq, wk, wv, wo, jin_kv, kv_len,
                          post_qc=None, kv_done=False):
                if not kv_done:
                    emit_kv(prefix, kv_src, wk, wv, jin_kv, kv_len)

                for qc in range(NQ):
                    PHASES.append((f'{prefix}:qc{qc}',
                                   int(__import__('re').findall(
                                       r'\d+', nc.get_next_instruction_name())[0])))
                    qs = slice(qc * QCH, (qc + 1) * QCH)
                    # Q^T for this q-chunk (scale folded into wq on host)
                    for of in range(JE):
                        wt = wload(wq, of, JE)
                        wchain(wt, JE, lambda j: lnt[j][:, qs],
                               QT[of][:], QCH,
                               vap(f'bq_{prefix}', of)
                               if flags[f'bq_{prefix}'] else None)
                    nkt = kv_len // 128
                    npair = nkt // 2
                    seq = [(h, kp) for h in range(H) for kp in range(npair)]
                    _sr = {'i': 0}
                    state = {}

                    def load_eb(h):
                        ebts = []
                        for kp in range(npair):
                            ebt = T(ebpool, [128, 2 * QCH], F16, "eb", bufs=8)
                            ring = nc.gpsimd if (h % 2 == 0) else nc.scalar
                            ring.dma_start(
                                ebt[:].rearrange("p (t c) -> p t c", t=2),
                                expb_d[h, kp * 256:(kp + 1) * 256, qs]
                                .rearrange("(t p) c -> p t c", p=128))
                            ebts.append(ebt)
                        state.setdefault(h, {'tiles': []})['ebts'] = ebts

                    load_eb(0)

                    def s_stage(i):
                        h, kp = seq[i]
                        st = state.setdefault(h, {'tiles': []})
                        if kp == 0 and h + 1 < H:
                            load_eb(h + 1)
                        th, ph = (h * D) // 128, (h * D) % 128
                        sc = scp_tile(_sr['i'] % 3)
                        _sr['i'] += 1
                        for half in range(2):
                            kvt = 2 * kp + half
                            chain(nc.tensor.matmul(
                                sc[:, half * QCH:(half + 1) * QCH],
                                KT[th][ph:ph + D, kvt * 128:(kvt + 1) * 128],
                                QT[th][ph:ph + D, :], start=True, stop=True))
                        pe = T(pepool, [128, 2 * QCH], F16, "pe", bufs=6)
                        nc.scalar.activation(pe[:], sc[:], AF.Exp)
                        nc.vector.tensor_tensor(pe[:], pe[:],
                                                st['ebts'][kp][:], op=OP.mult)
                        st['tiles'].append(pe)

                    def pv_stage(i):
                        h, kp = seq[i]
                        st = state[h]
                        th, ph = (h * D) // 128, (h * D) % 128
                        if kp == 0:
                            st['pv'] = psum_tile(f"pv{h % 2}")
                        pe = st['tiles'][kp]
                        for half in range(2):
                            kvt = 2 * kp + half
                            chain(nc.tensor.matmul(
                                st['pv'][:], Vx[kvt][:, h * 128:(h + 1) * 128],
                                pe[:, half * QCH:(half + 1) * QCH],
                                start=(kvt == 0), stop=(kvt == nkt - 1)))
                        if kp == npair - 1:
                            pv = st['pv']

                            def fin(pv=pv, th=th, ph=ph):
                                # psum rows 0:64 = kv-sums, 64:128 = P@V
                                rec = T(tr, [64, QCH], F32, "rec", bufs=2)
                                nc.vector.reciprocal_approx_fast(
                                    rec[:], pv[0:64, :])
                                nc.vector.tensor_tensor(AT[th][ph:ph + D, :],
                                                        pv[64:128, :],
                                                        rec[:], op=OP.mult)
                            deferred.append(fin)
                            del state[h]

                    _pe_chain = {'prev': None}

                    def chain(bi):
                        if _pe_chain['prev'] is not None:
                            add_dep_helper(bi.ins, _pe_chain['prev'].ins,
                                           sync=False, reason="pe-order")
                        _pe_chain['prev'] = bi

                    deferred = []
                    BLK = 3
                    blocks = [list(range(i, min(i + BLK, len(seq))))
                              for i in range(0, len(seq), BLK)]
                    for j in range(len(blocks) + 1):
                        if j < len(blocks):
                            for i in blocks[j]:
                                s_stage(i)
                        while len(deferred) > 1:
                            deferred.pop(0)()
                        if j >= 1:
                            for i in blocks[j - 1]:
                                pv_stage(i)
                    while deferred:
                        deferred.pop(0)()
                    # out-projection + residual
                    for of in range(JE):
                        wt = wload(wo, of, JE)
                        pt = rot_ps()
                        for j in range(JE):
                            nc.tensor.matmul(pt, wt[:, j * 128:(j + 1) * 128],
                                             AT[j][:],
                                             start=(j == 0), stop=(j == JE - 1))
                        if flags[f'bo_{prefix}']:
                            nc.vector.scalar_tensor_tensor(
                                res_out[of][:, qs], pt, vap(f'bo_{prefix}', of),
                                res_in[of][:, qs], op0=OP.add, op1=OP.add)
                        else:
                            nc.vector.tensor_tensor(res_out[of][:, qs], pt,
                                                    res_in[of][:, qs], op=OP.add)
                    if post_qc is not None:
                        post_qc(qc)

            # ================= the layer =================
            import re as _re

            def _mark(lbl):
                n = int(_re.findall(r'\d+', nc.get_next_instruction_name())[0])
                PHASES.append((lbl, n))

            _mark('ckv')
            # cross K/V only need ctxT: emit first to overlap xT DMA
            emit_kv('c', ctxT, w_d['wk_c'], w_d['wv_c'], JC, L)
            _mark('ln1')
            ln_phase(rA, lnT, 'cn_g', 'cn_b', flags['cn'])
            _mark('cross')
            attention('c', lnT, ctxT, expb_c_d, rA, rB,
                      w_d['wq_c'], w_d['wk_c'], w_d['wv_c'], w_d['wo_c'],
                      JC, L, kv_done=True,
                      post_qc=lambda qc: ln_phase(rB, lnT, 'sn_g', 'sn_b',
                                                  flags['sn'], only_qc=qc))
            _mark('self')
            attention('s', lnT, lnT, expb_s_d, rB, rA,
                      w_d['wq_s'], w_d['wk_s'], w_d['wv_s'], w_d['wo_s'],
                      JE, S,
                      post_qc=lambda qc: ln_phase(rA, lnT, 'fn_g', 'fn_b',
                                                  flags['fn'], only_qc=qc))
            _mark('ffn')

            # ================= FFN =================
            for qc in range(NQ):
                qs = slice(qc * QCH, (qc + 1) * QCH)
                ytiles = [scp_tile(k) for k in range(3)]
                ypt = [ytiles[k // 2][:, (k % 2) * QCH:(k % 2 + 1) * QCH]
                       for k in range(JE)]

                def emit_f1(of):
                    wt = T(wst, [128, JE * 128], F16, "w1g", bufs=4)
                    nc.sync.dma_start(wt[:], w_d['w1'][of])
                    f1 = psum_tile(f"pv{of % 2}")
                    for j in range(JE):
                        nc.tensor.matmul(f1[:], wt[:, j * 128:(j + 1) * 128],
                                         lnT[j][:, qs],
                                         start=(j == 0), stop=(j == JE - 1))
                    g = T(tr, [128, QCH], F16, "gelu", bufs=3)
                    nc.scalar.activation(g[:], f1[:], AF.Gelu_apprx_tanh,
                                         bias=vap('b1', of) if flags['b1'] else 0.0)
                    return g

                def load_w2(of):
                    w2t = T(wst, [128, JE * 128], F16, "w2g", bufs=4)
                    nc.gpsimd.dma_start(w2t[:], w_d['w2'][of])
                    return w2t

                gprev = emit_f1(0)
                w2prev = load_w2(0)
                for of in range(JF):
                    gnext = emit_f1(of + 1) if of + 1 < JF else None
                    w2next = load_w2(of + 1) if of + 1 < JF else None
                    for of2 in range(JE):
                        nc.tensor.matmul(ypt[of2],
                                         w2prev[:, of2 * 128:(of2 + 1) * 128],
                                         gprev[:],
                                         start=(of == 0), stop=(of == JF - 1))
                    gprev = gnext
                    w2prev = w2next
                for of2 in range(JE):
                    yo = T(tr, [128, QCH], F32, "yout", bufs=6)
                    if flags['b2']:
                        nc.vector.tensor_scalar(yo[:], ypt[of2], vap('b2', of2),
                                                None, op0=OP.add)
                    else:
                        nc.vector.tensor_copy(yo[:], ypt[of2])
                    ring = [nc.sync, nc.scalar, nc.gpsimd][of2 % 3]
                    ring.dma_start(yT_d[of2 * 128:(of2 + 1) * 128, qs], yo[:])

    nc.compile()
    return nc


def kernel(**inputs):
    inp = {k: np.asarray(v, dtype=np.float32) for k, v in inputs.items()}
    triv1 = lambda v: bool(np.all(v == 1.0))
    triv0 = lambda v: bool(np.all(v == 0.0))
    flags = {
        'cn': not (triv1(inp['cn_g']) and triv0(inp['cn_b'])),
        'sn': not (triv1(inp['sn_g']) and triv0(inp['sn_b'])),
        'fn': not (triv1(inp['fn_g']) and triv0(inp['fn_b'])),
        'bq_c': not triv0(inp['bq_c']), 'bk_c': not triv0(inp['bk_c']),
        'bo_c': not triv0(inp['bo_c']), 'bq_s': not triv0(inp['bq_s']),
        'bk_s': not triv0(inp['bk_s']), 'bo_s': not triv0(inp['bo_s']),
        'b1': not triv0(inp['b1']), 'b2': not triv0(inp['b2']),
    }
    assert triv0(inp['bv_c']) and triv0(inp['bv_s']), \
        "nonzero V bias not supported by this build"

    key = tuple(sorted(flags.items()))
    if key not in _BUILT:
        _BUILT[key] = _build(flags)
    nc = _BUILT[key]

    from concourse.bass_utils import run_bass_kernel_spmd

    f16 = np.float16
    scale = 1.0 / np.sqrt(np.float32(D))

    def tile_w(W, jin, ofn):
        return np.ascontiguousarray(
            W.reshape(jin, 128, ofn, 128).transpose(2, 1, 0, 3)
            .reshape(ofn, 128, jin * 128).astype(f16))

    def tile_v(W, jin):
        return np.ascontiguousarray(W.reshape(jin, 128, E).astype(f16))

    com = {
        'wq_c': tile_w(inp['wq_c'] * scale, JE, JE),
        'wk_c': tile_w(inp['wk_c'], JC, JE),
        'wv_c': tile_v(inp['wv_c'], JC),
        'wo_c': tile_w(inp['wo_c'], JE, JE),
        'wq_s': tile_w(inp['wq_s'] * scale, JE, JE),
        'wk_s': tile_w(inp['wk_s'], JE, JE),
        'wv_s': tile_v(inp['wv_s'], JE),
        'wo_s': tile_w(inp['wo_s'], JE, JE),
        'w1': tile_w(inp['w1'], JE, JF),
        'w2': np.ascontiguousarray(inp['w2'].reshape(JF, 128, E).astype(f16)),
        'expb_c': np.ascontiguousarray(
            np.exp(inp['bias_c'].transpose(0, 2, 1)).astype(f16)),
        'expb_s': np.ascontiguousarray(
            np.exp(inp['bias_s'].transpose(0, 2, 1)).astype(f16)),
    }
    chunks = []
    for nm in ['cn_g', 'cn_b', 'sn_g', 'sn_b', 'fn_g', 'fn_b']:
        chunks.append(inp[nm].reshape(-1, 128))
    chunks.append((inp['bq_c'] * scale).reshape(-1, 128))
    for nm in ['bk_c', 'bo_c']:
        chunks.append(inp[nm].reshape(-1, 128))
    chunks.append((inp['bq_s'] * scale).reshape(-1, 128))
    for nm in ['bk_s', 'bo_s', 'b1', 'b2']:
        chunks.append(inp[nm].reshape(-1, 128))
    com['vecs'] = np.ascontiguousarray(np.concatenate(chunks, 0).T)

    in_maps = []
    for b in range(B):
        m = dict(com)
        m['xT'] = np.ascontiguousarray(inp['hidden_state'][b].T)
        m['ctxT'] = np.ascontiguousarray(inp['context'][b].T.astype(f16))
        in_maps.append(m)

    res = run_bass_kernel_spmd(nc, in_maps, core_ids=list(range(NCORES)),
                               trace=TRACE)
    LAST['res'] = res
    y = np.stack([res.results[c]['yT'].T for c in range(B)])
    return np.ascontiguousarray(y.astype(np.float32))


# revision 17
# speedup vs baseline: 1.0252x; 1.0252x over previous
"""Trainium2 Bass kernel for nn_BasicTransformerLayer (dense transformer layer).

Strategy v2:
- Data-parallel over batch: B=8, one batch element per NeuronCore, no
  collectives.
- Activations transposed [features, tokens]; residual stream in fp32r,
  everything on matmul paths in float16 (weights pre-tiled on host into
  contiguous per-output-tile blocks for max DMA efficiency).
- Softmax: exp(scores) on scalar engine (f16), multiplicative exp(bias)
  (host-precomputed, f16) on vector; the kv-sum (softmax denominator) is
  folded into the P@V matmul via V tiles laid out [64 ones | 64 values]
  per head: psum rows 0:64 = sums, 64:128 = P@V.
- DMA spread across sync/scalar/gpsimd rings; output written per-psum-bank
  with rotating buffers to avoid a serialized tail.
"""
import sys

sys.path.insert(0, '/opt/trn_rl_repo')

import numpy as np

E, C, H, D, FF = 768, 512, 12, 64, 3072
B, S, L = 8, 1024, 256
EPS = 1e-5
NCORES = 8
QCH = 512                  # q-chunk (matmul moving free dim)
NQ = S // QCH              # 2
JE = E // 128              # 6 feature tiles
JC = C // 128              # 4 cross-feature tiles
JF = FF // 128             # 24 ffn tiles
KVS = S // 128             # 8 self kv tiles
KVC = L // 128             # 2 cross kv tiles

_BUILT = {}
TRACE = False
LAST = {}
PHASES = []


def _build(flags):
    import concourse.bacc as bacc
    import concourse.mybir as mybir
    import concourse.tile as tile
    from concourse.tile import add_dep_helper
    from concourse.bass import AP as BassAP

    FR = mybir.dt.float32r
    F32 = mybir.dt.float32
    F16 = mybir.dt.float16
    AF = mybir.ActivationFunctionType
    OP = mybir.AluOpType

    nc = bacc.Bacc("TRN2", target_bir_lowering=False, debug=False,
                   enable_asserts=True, num_devices=NCORES)

    def din(name, shape, dt=F16):
        return nc.dram_tensor(name, shape, dt, kind="ExternalInput").ap()

    xT_d = din("xT", [E, S], F32)
    ctxT_d = din("ctxT", [C, L])
    # pre-tiled weights: [ofn, 128, jin*128] f16 (contiguous per of)
    w_d = {
        'wq_c': din("wq_c", [JE, 128, JE * 128]),
        'wk_c': din("wk_c", [JE, 128, JC * 128]),
        'wo_c': din("wo_c", [JE, 128, JE * 128]),
        'wq_s': din("wq_s", [JE, 128, JE * 128]),
        'wk_s': din("wk_s", [JE, 128, JE * 128]),
        'wo_s': din("wo_s", [JE, 128, JE * 128]),
        'w1': din("w1", [JF, 128, JE * 128]),
        'w2': din("w2", [JF, 128, JE * 128]),
        # V weights: [jin, 128, E] (row blocks of original)
        'wv_c': din("wv_c", [JC, 128, E]),
        'wv_s': din("wv_s", [JE, 128, E]),
    }
    expb_c_d = din("expb_c", [H, L, S])
    expb_s_d = din("expb_s", [H, S, S])
    VIDX = {}
    _off = 0
    for nm, ln in [('cn_g', JE), ('cn_b', JE), ('sn_g', JE), ('sn_b', JE),
                   ('fn_g', JE), ('fn_b', JE), ('bq_c', JE), ('bk_c', JE),
                   ('bo_c', JE), ('bq_s', JE), ('bk_s', JE), ('bo_s', JE),
                   ('b1', JF), ('b2', JE)]:
        VIDX[nm] = _off
        _off += ln
    NV = _off
    vecs_d = din("vecs", [128, NV], F32)
    yT_d = nc.dram_tensor("yT", [E, S], F32, kind="ExternalOutput").ap()

    with tile.TileContext(nc) as tc:
        with tc.tile_pool(name="const", bufs=1) as cpool, \
             tc.tile_pool(name="acts", bufs=1) as acts, \
             tc.tile_pool(name="wst", bufs=8) as wst, \
             tc.tile_pool(name="tr", bufs=2) as tr, \
             tc.tile_pool(name="pe", bufs=4) as pepool, \
             tc.tile_pool(name="eb", bufs=4) as ebpool, \
             tc.tile_pool(name="ps", bufs=1, space="PSUM") as ps:

            def T(pool, shape, dtype, tag, bufs=1):
                return pool.tile(shape, dtype, tag=tag, name=tag, bufs=bufs)

            ones = T(cpool, [128, 128], FR, "ones")
            ones16 = T(cpool, [128, 128], F16, "ones16")
            ones_f = T(cpool, [128, 128], F32, "ones_f")
            epsc = T(cpool, [128, 1], F32, "epsc")
            nc.vector.memset(epsc[:], EPS)
            nc.vector.memset(ones_f[:], 1.0)
            nc.vector.tensor_copy(ones[:], ones_f[:])
            nc.vector.tensor_copy(ones16[:], ones_f[:])
            vecs = T(cpool, [128, NV], F32, "vecs")
            nc.sync.dma_start(vecs[:], vecs_d[:])

            def vap(nm, j):
                return vecs[:, VIDX[nm] + j:VIDX[nm] + j + 1]

            # persistent activation tiles
            rA = [T(acts, [128, S], FR, f"rA{j}") for j in range(JE)]
            rB = [T(acts, [128, S], FR, f"rB{j}") for j in range(JE)]
            lnT = [T(acts, [128, S], F16, f"ln{j}") for j in range(JE)]
            KT = [T(acts, [128, S], F16, f"KT{j}") for j in range(JE)]
            # V tiles: per head 128 cols = [64 ones | 64 values]
            Vx = [T(acts, [128, H * 128], F16, f"V{t}") for t in range(KVS)]
            QT = [T(acts, [128, QCH], F16, f"QT{j}") for j in range(JE)]
            AT = [T(acts, [128, QCH], F16, f"AT{j}") for j in range(JE)]
            ctxT = [T(acts, [128, L], F16, f"cx{j}") for j in range(JC)]

            def vx_strided(t, head0, nh, ones_cols):
                vp = Vx[t][:]
                pstride = vp.ap[0][0]
                off = vp.offset + head0 * 128 + (0 if ones_cols else 64)
                return BassAP(vp.tensor, off,
                              [[pstride, 128], [128, nh], [1, 64]])

            for t in range(KVS):
                nc.vector.memset(vx_strided(t, 0, H, True), 1.0)

            for j in range(JC):
                nc.sync.dma_start(ctxT[j][:], ctxT_d[j * 128:(j + 1) * 128, :])

            def load_xT():
                # emitted after cross-K/V weight DMAs: ring order = emission
                # order, so K/V weights land first and the PE starts at once
                for half in range(2):
                    cs_ = slice(half * QCH, (half + 1) * QCH)
                    for j in range(JE):
                        ring = nc.sync if j % 2 == 0 else nc.scalar
                        ring.dma_start(
                            rA[j][:, cs_],
                            xT_d[j * 128:(j + 1) * 128, cs_].bitcast(FR))

            # PSUM: 3x double-bank "scp" tiles (scores/general) + 2 single
            # "pv" banks = 8 banks total.
            def psum_tile(tag, n=QCH):
                return ps.tile([128, n], F32, tag=tag, name=tag)

            def scp_tile(k):
                return psum_tile(f"scp{k % 3}", 2 * QCH)

            _rot = {'i': 0, 'cur': None}

            def rot_ps(n=QCH):
                i = _rot['i']
                _rot['i'] += 1
                if i % 2 == 0:
                    _rot['cur'] = scp_tile(i // 2)
                return _rot['cur'][:, (i % 2) * QCH:(i % 2) * QCH + n]

            # ---------------- layer norm (transposed layout) ----------------
            def ln_phase(src, dst, gname, bname, affine, only_qc=None):
                inv = 1.0 / float(E)
                for qc in range(NQ):
                    if only_qc is not None and qc != only_qc:
                        continue
                    qs = slice(qc * QCH, (qc + 1) * QCH)
                    sqs = []
                    for j in range(JE):
                        sq = T(tr, [128, QCH], F16, "sq", bufs=6)
                        nc.scalar.activation(sq[:], src[j][:, qs], AF.Square)
                        sqs.append(sq)
                    s12 = scp_tile(0)
                    s1 = s12[:, 0:QCH]
                    s2 = s12[:, QCH:2 * QCH]
                    for j in range(JE):
                        nc.tensor.matmul(s1, ones[:, 0:128], src[j][:, qs],
                                         start=(j == 0), stop=(j == JE - 1))
                    for j in range(JE):
                        nc.tensor.matmul(s2, ones16[:, 0:128], sqs[j][:],
                                         start=(j == 0), stop=(j == JE - 1))
                    t1 = T(tr, [128, QCH], F32, "t1m", bufs=1)
                    nc.scalar.activation(t1[:], s1[:], AF.Square, scale=inv)
                    var = T(tr, [128, QCH], F32, "var", bufs=1)
                    nc.vector.scalar_tensor_tensor(var[:], s2[:], inv, t1[:],
                                                   op0=OP.mult, op1=OP.subtract)
                    nc.scalar.activation(var[:], var[:], AF.Sqrt,
                                         bias=epsc[:, 0:1])
                    rstd = T(tr, [128, QCH], F32, "rstd", bufs=1)
                    nc.vector.reciprocal_approx_fast(rstd[:], var[:])
                    m1r = T(tr, [128, QCH], F32, "m1r", bufs=1)
                    nc.vector.scalar_tensor_tensor(m1r[:], s1[:], inv, rstd[:],
                                                   op0=OP.mult, op1=OP.mult)
                    for j in range(JE):
                        tmp = T(tr, [128, QCH], F32, "lntmp", bufs=2)
                        nc.vector.tensor_tensor(tmp[:], src[j][:, qs], rstd[:],
                                                op=OP.mult)
                        if affine:
                            tmp2 = T(tr, [128, QCH], F32, "lntmp2", bufs=2)
                            nc.vector.tensor_tensor(tmp2[:], tmp[:], m1r[:],
                                                    op=OP.subtract)
                            nc.vector.tensor_scalar(dst[j][:, qs], tmp2[:],
                                                    vap(gname, j), vap(bname, j),
                                                    op0=OP.mult, op1=OP.add)
                        else:
                            nc.vector.tensor_tensor(dst[j][:, qs], tmp[:],
                                                    m1r[:], op=OP.subtract)

            # -------- projection from pre-tiled weights --------
            def wload(wd, of, jin, ring=None):
                wt = T(wst, [128, JE * 128], F16, "wg", bufs=6)
                (ring or nc.sync).dma_start(wt[:, 0:jin * 128], wd[of])
                return wt

            _cpn = {'i': 0}

            def wchain(wt, jin, src_getter, out_ap, n, bias_ap, pt=None):
                if pt is None:
                    pt = rot_ps(n)
                for j in range(jin):
                    nc.tensor.matmul(pt, wt[:, j * 128:(j + 1) * 128],
                                     src_getter(j),
                                     start=(j == 0), stop=(j == jin - 1))
                if bias_ap is not None:
                    nc.vector.tensor_scalar(out_ap, pt, bias_ap, None,
                                            op0=OP.add)
                elif _cpn['i'] % 2 == 0:
                    _cpn['i'] += 1
                    nc.scalar.copy(out_ap, pt)
                else:
                    _cpn['i'] += 1
                    nc.vector.tensor_copy(out_ap, pt)

            # ---------------- K/V projection emission ----------------
            def emit_kv(prefix, kv_src, wk, wv, jin_kv, kv_len):
                nkv = kv_len // 128

                _kps = {'i': 0}

                def emit_k(of):
                    wt = wload(wk, of, jin_kv)
                    for ks in range(0, kv_len, QCH):
                        n = min(QCH, kv_len - ks)
                        kp = psum_tile(f"pv{_kps['i'] % 2}")
                        _kps['i'] += 1
                        wchain(wt, jin_kv,
                               lambda j: kv_src[j][:, ks:ks + n],
                               KT[of][:, ks:ks + n], n,
                               vap(f'bk_{prefix}', of)
                               if flags[f'bk_{prefix}'] else None,
                               pt=kp[:, 0:n])

                def emit_vgroup(os_, tg):
                    n = min(QCH, E - os_)
                    tcnt = min(4, nkv - tg)
                    vts = [scp_tile(1), scp_tile(2)]
                    vps = [vts[i // 2][:, (i % 2) * QCH:(i % 2) * QCH + n]
                           for i in range(tcnt)]
                    for j in range(jin_kv):
                        wt = T(wst, [128, QCH], F16, "wv", bufs=3)
                        nc.sync.dma_start(wt[:, 0:n], wv[j, :, os_:os_ + n])
                        for i in range(tcnt):
                            nc.tensor.matmul(
                                vps[i][:, 0:n],
                                kv_src[j][:, (tg + i) * 128:(tg + i + 1) * 128],
                                wt[:, 0:n], start=(j == 0),
                                stop=(j == jin_kv - 1))
                    for i in range(tcnt):
                        dst = vx_strided(tg + i, os_ // 64, n // 64, False)
                        src = vps[i][:, 0:n].rearrange("p (h d) -> p h d", d=64)
                        nc.scalar.copy(dst, src)

                vgroups = [(os_, tg) for os_ in range(0, E, QCH)
                           for tg in range(0, nkv, 4)]
                for i in range(max(JE, len(vgroups))):
                    if i < JE:
                        emit_k(i)
                    if i < len(vgroups):
                        emit_vgroup(*vgroups[i])

            # ---------------- attention (shared cross/self) ----------------
            def attention(prefix, lnt, kv_src, expb_d, res_in,
                          res_out, wq, wk, wv, wo, jin_kv, kv_len,
                          post_qc=None, kv_done=False):
                if not kv_done:
                    emit_kv(prefix, kv_src, wk, wv, jin_kv, kv_len)

                for qc in range(NQ):
                    PHASES.append((f'{prefix}:qc{qc}',
                                   int(__import__('re').findall(
                                       r'\d+', nc.get_next_instruction_name())[0])))
                    qs = slice(qc * QCH, (qc + 1) * QCH)
                    # Q^T for this q-chunk (scale folded into wq on host)
                    for of in range(JE):
                        wt = wload(wq, of, JE)
                        wchain(wt, JE, lambda j: lnt[j][:, qs],
                               QT[of][:], QCH,
                               vap(f'bq_{prefix}', of)
                               if flags[f'bq_{prefix}'] else None)
                    nkt = kv_len // 128
                    npair = nkt // 2
                    seq = [(h, kp) for h in range(H) for kp in range(npair)]
                    _sr = {'i': 0}
                    state = {}

                    def load_eb(h):
                        ebts = []
                        for kp in range(npair):
                            ebt = T(ebpool, [128, 2 * QCH], F16, "eb", bufs=8)
                            ring = nc.gpsimd if (h % 2 == 0) else nc.scalar
                            ring.dma_start(
                                ebt[:].rearrange("p (t c) -> p t c", t=2),
                                expb_d[h, kp * 256:(kp + 1) * 256, qs]
                                .rearrange("(t p) c -> p t c", p=128))
                            ebts.append(ebt)
                        state.setdefault(h, {'tiles': []})['ebts'] = ebts

                    load_eb(0)

                    def s_stage(i):
                        h, kp = seq[i]
                        st = state.setdefault(h, {'tiles': []})
                        if kp == 0 and h + 1 < H:
                            load_eb(h + 1)
                        th, ph = (h * D) // 128, (h * D) % 128
                        sc = scp_tile(_sr['i'] % 3)
                        _sr['i'] += 1
                        for half in range(2):
                            kvt = 2 * kp + half
                            chain(nc.tensor.matmul(
                                sc[:, half * QCH:(half + 1) * QCH],
                                KT[th][ph:ph + D, kvt * 128:(kvt + 1) * 128],
                                QT[th][ph:ph + D, :], start=True, stop=True))
                        pe = T(pepool, [128, 2 * QCH], F16, "pe", bufs=6)
                        nc.scalar.activation(pe[:], sc[:], AF.Exp)
                        nc.vector.tensor_tensor(pe[:], pe[:],
                                                st['ebts'][kp][:], op=OP.mult)
                        st['tiles'].append(pe)

                    def pv_stage(i):
                        h, kp = seq[i]
                        st = state[h]
                        th, ph = (h * D) // 128, (h * D) % 128
                        if kp == 0:
                            st['pv'] = psum_tile(f"pv{h % 2}")
                        pe = st['tiles'][kp]
                        for half in range(2):
                            kvt = 2 * kp + half
                            chain(nc.tensor.matmul(
                                st['pv'][:], Vx[kvt][:, h * 128:(h + 1) * 128],
                                pe[:, half * QCH:(half + 1) * QCH],
                                start=(kvt == 0), stop=(kvt == nkt - 1)))
                        if kp == npair - 1:
                            pv = st['pv']

                            def fin(pv=pv, th=th, ph=ph):
                                # psum rows 0:64 = kv-sums, 64:128 = P@V
                                rec = T(tr, [64, QCH], F32, "rec", bufs=2)
                                nc.vector.reciprocal_approx_fast(
                                    rec[:], pv[0:64, :])
                                nc.vector.tensor_tensor(AT[th][ph:ph + D, :],
                                                        pv[64:128, :],
                                                        rec[:], op=OP.mult)
                            deferred.append(fin)
                            del state[h]

                    _pe_chain = {'prev': None}

                    def chain(bi):
                        if _pe_chain['prev'] is not None:
                            add_dep_helper(bi.ins, _pe_chain['prev'].ins,
                                           sync=False, reason="pe-order")
                        _pe_chain['prev'] = bi

                    deferred = []
                    BLK = 3
                    blocks = [list(range(i, min(i + BLK, len(seq))))
                              for i in range(0, len(seq), BLK)]
                    for j in range(len(blocks) + 1):
                        if j < len(blocks):
                            for i in blocks[j]:
                                s_stage(i)
                        while len(deferred) > 1:
                            deferred.pop(0)()
                        if j >= 1:
                            for i in blocks[j - 1]:
                                pv_stage(i)
                    while deferred:
                        deferred.pop(0)()
                    # out-projection + residual
                    for of in range(JE):
                        wt = wload(wo, of, JE)
                        pt = rot_ps()
                        for j in range(JE):
                            nc.tensor.matmul(pt, wt[:, j * 128:(j + 1) * 128],
                                             AT[j][:],
                                             start=(j == 0), stop=(j == JE - 1))
                        if flags[f'bo_{prefix}']:
                            nc.vector.scalar_tensor_tensor(
                                res_out[of][:, qs], pt, vap(f'bo_{prefix}', of),
                                res_in[of][:, qs], op0=OP.add, op1=OP.add)
                        else:
                            nc.vector.tensor_tensor(res_out[of][:, qs], pt,
                                                    res_in[of][:, qs], op=OP.add)
                    if post_qc is not None:
                        post_qc(qc)

            # ================= the layer =================
            import re as _re

            def _mark(lbl):
                n = int(_re.findall(r'\d+', nc.get_next_instruction_name())[0])
                PHASES.append((lbl, n))

            _mark('ckv')
            # cross K/V only need ctxT: emit first to overlap xT DMA
            emit_kv('c', ctxT, w_d['wk_c'], w_d['wv_c'], JC, L)
            load_xT()
            _mark('ln1')
            ln_phase(rA, lnT, 'cn_g', 'cn_b', flags['cn'])
            _mark('cross')
            attention('c', lnT, ctxT, expb_c_d, rA, rB,
                      w_d['wq_c'], w_d['wk_c'], w_d['wv_c'], w_d['wo_c'],
                      JC, L, kv_done=True,
                      post_qc=lambda qc: ln_phase(rB, lnT, 'sn_g', 'sn_b',
                                                  flags['sn'], only_qc=qc))
            _mark('self')
            attention('s', lnT, lnT, expb_s_d, rB, rA,
                      w_d['wq_s'], w_d['wk_s'], w_d['wv_s'], w_d['wo_s'],
                      JE, S,
                      post_qc=lambda qc: ln_phase(rA, lnT, 'fn_g', 'fn_b',
                                                  flags['fn'], only_qc=qc))
            _mark('ffn')

            # ================= FFN =================
            for qc in range(NQ):
                qs = slice(qc * QCH, (qc + 1) * QCH)
                ytiles = [scp_tile(k) for k in range(3)]
                ypt = [ytiles[k // 2][:, (k % 2) * QCH:(k % 2 + 1) * QCH]
                       for k in range(JE)]

                def emit_f1(of):
                    wt = T(wst, [128, JE * 128], F16, "w1g", bufs=4)
                    nc.sync.dma_start(wt[:], w_d['w1'][of])
                    f1 = psum_tile(f"pv{of % 2}")
                    for j in range(JE):
                        nc.tensor.matmul(f1[:], wt[:, j * 128:(j + 1) * 128],
                                         lnT[j][:, qs],
                                         start=(j == 0), stop=(j == JE - 1))
                    g = T(tr, [128, QCH], F16, "gelu", bufs=3)
                    nc.scalar.activation(g[:], f1[:], AF.Gelu_apprx_tanh,
                                         bias=vap('b1', of) if flags['b1'] else 0.0)
                    return g

                def load_w2(of):
                    w2t = T(wst, [128, JE * 128], F16, "w2g", bufs=4)
                    nc.gpsimd.dma_start(w2t[:], w_d['w2'][of])
                    return w2t

                gprev = emit_f1(0)
                w2prev = load_w2(0)
                for of in range(JF):
                    gnext = emit_f1(of + 1) if of + 1 < JF else None
                    w2next = load_w2(of + 1) if of + 1 < JF else None
                    for of2 in range(JE):
                        nc.tensor.matmul(ypt[of2],
                                         w2prev[:, of2 * 128:(of2 + 1) * 128],
                                         gprev[:],
                                         start=(of == 0), stop=(of == JF - 1))
                    gprev = gnext
                    w2prev = w2next
                for of2 in range(JE):
                    yo = T(tr, [128, QCH], F32, "yout", bufs=6)
                    if flags['b2']:
                        nc.vector.tensor_scalar(yo[:], ypt[of2], vap('b2', of2),
                                                None, op0=OP.add)
                    else:
                        nc.vector.tensor_copy(yo[:], ypt[of2])
                    ring = [nc.sync, nc.scalar][of2 % 2]
                    ring.dma_start(yT_d[of2 * 128:(of2 + 1) * 128, qs], yo[:])

    nc.compile()
    return nc


def kernel(**inputs):
    inp = {k: np.asarray(v, dtype=np.float32) for k, v in inputs.items()}
    triv1 = lambda v: bool(np.all(v == 1.0))
    triv0 = lambda v: bool(np.all(v == 0.0))
    flags = {
        'cn': not (triv1(inp['cn_g']) and triv0(inp['cn_b'])),
        'sn': not (triv1(inp['sn_g']) and triv0(inp['sn_b'])),
        'fn': not (triv1(inp['fn_g']) and triv0(inp['fn_b'])),
        'bq_c': not triv0(inp['bq_c']), 'bk_c': not triv0(inp['bk_c']),
        'bo_c': not triv0(inp['bo_c']), 'bq_s': not triv0(inp['bq_s']),
        'bk_s': not triv0(inp['bk_s']), 'bo_s': not triv0(inp['bo_s']),
        'b1': not triv0(inp['b1']), 'b2': not triv0(inp['b2']),
    }
    assert triv0(inp['bv_c']) and triv0(inp['bv_s']), \
        "nonzero V bias not supported by this build"

    key = tuple(sorted(flags.items()))
    if key not in _BUILT:
        _BUILT[key] = _build(flags)
    nc = _BUILT[key]

    from concourse.bass_utils import run_bass_kernel_spmd

    f16 = np.float16
    scale = 1.0 / np.sqrt(np.float32(D))

    def tile_w(W, jin, ofn):
        return np.ascontiguousarray(
            W.reshape(jin, 128, ofn, 128).transpose(2, 1, 0, 3)
            .reshape(ofn, 128, jin * 128).astype(f16))

    def tile_v(W, jin):
        return np.ascontiguousarray(W.reshape(jin, 128, E).astype(f16))

    com = {
        'wq_c': tile_w(inp['wq_c'] * scale, JE, JE),
        'wk_c': tile_w(inp['wk_c'], JC, JE),
        'wv_c': tile_v(inp['wv_c'], JC),
        'wo_c': tile_w(inp['wo_c'], JE, JE),
        'wq_s': tile_w(inp['wq_s'] * scale, JE, JE),
        'wk_s': tile_w(inp['wk_s'], JE, JE),
        'wv_s': tile_v(inp['wv_s'], JE),
        'wo_s': tile_w(inp['wo_s'], JE, JE),
        'w1': tile_w(inp['w1'], JE, JF),
        'w2': np.ascontiguousarray(inp['w2'].reshape(JF, 128, E).astype(f16)),
        'expb_c': np.ascontiguousarray(
            np.exp(inp['bias_c'].transpose(0, 2, 1)).astype(f16)),
        'expb_s': np.ascontiguousarray(
            np.exp(inp['bias_s'].transpose(0, 2, 1)).astype(f16)),
    }
    chunks = []
    for nm in ['cn_g', 'cn_b', 'sn_g', 'sn_b', 'fn_g', 'fn_b']:
        chunks.append(inp[nm].reshape(-1, 128))
    chunks.append((inp['bq_c'] * scale).reshape(-1, 128))
    for nm in ['bk_c', 'bo_c']:
        chunks.append(inp[nm].reshape(-1, 128))
    chunks.append((inp['bq_s'] * scale).reshape(-1, 128))
    for nm in ['bk_s', 'bo_s', 'b1', 'b2']:
        chunks.append(inp[nm].reshape(-1, 128))
    com['vecs'] = np.ascontiguousarray(np.concatenate(chunks, 0).T)

    in_maps = []
    for b in range(B):
        m = dict(com)
        m['xT'] = np.ascontiguousarray(inp['hidden_state'][b].T)
        m['ctxT'] = np.ascontiguousarray(inp['context'][b].T.astype(f16))
        in_maps.append(m)

    res = run_bass_kernel_spmd(nc, in_maps, core_ids=list(range(NCORES)),
                               trace=TRACE)
    LAST['res'] = res
    y = np.stack([res.results[c]['yT'].T for c in range(B)])
    return np.ascontiguousarray(y.astype(np.float32))


# revision 18
# speedup vs baseline: 1.0443x; 1.0187x over previous
"""Trainium2 Bass kernel for nn_BasicTransformerLayer (dense transformer layer).

Strategy v2:
- Data-parallel over batch: B=8, one batch element per NeuronCore, no
  collectives.
- Activations transposed [features, tokens]; residual stream in fp32r,
  everything on matmul paths in float16 (weights pre-tiled on host into
  contiguous per-output-tile blocks for max DMA efficiency).
- Softmax: exp(scores) on scalar engine (f16), multiplicative exp(bias)
  (host-precomputed, f16) on vector; the kv-sum (softmax denominator) is
  folded into the P@V matmul via V tiles laid out [64 ones | 64 values]
  per head: psum rows 0:64 = sums, 64:128 = P@V.
- DMA spread across sync/scalar/gpsimd rings; output written per-psum-bank
  with rotating buffers to avoid a serialized tail.
"""
import sys

sys.path.insert(0, '/opt/trn_rl_repo')

import numpy as np

E, C, H, D, FF = 768, 512, 12, 64, 3072
B, S, L = 8, 1024, 256
EPS = 1e-5
NCORES = 8
QCH = 512                  # q-chunk (matmul moving free dim)
NQ = S // QCH              # 2
JE = E // 128              # 6 feature tiles
JC = C // 128              # 4 cross-feature tiles
JF = FF // 128             # 24 ffn tiles
KVS = S // 128             # 8 self kv tiles
KVC = L // 128             # 2 cross kv tiles

_BUILT = {}
TRACE = False
LAST = {}
PHASES = []


def _build(flags):
    import concourse.bacc as bacc
    import concourse.mybir as mybir
    import concourse.tile as tile
    from concourse.tile import add_dep_helper
    from concourse.bass import AP as BassAP

    FR = mybir.dt.float32r
    F32 = mybir.dt.float32
    F16 = mybir.dt.float16
    AF = mybir.ActivationFunctionType
    OP = mybir.AluOpType

    nc = bacc.Bacc("TRN2", target_bir_lowering=False, debug=False,
                   enable_asserts=True, num_devices=NCORES)

    def din(name, shape, dt=F16):
        return nc.dram_tensor(name, shape, dt, kind="ExternalInput").ap()

    xT_d = din("xT", [E, S], F32)
    ctxT_d = din("ctxT", [C, L])
    # pre-tiled weights: [ofn, 128, jin*128] f16 (contiguous per of)
    w_d = {
        'wq_c': din("wq_c", [JE, 128, JE * 128]),
        'wk_c': din("wk_c", [JE, 128, JC * 128]),
        'wo_c': din("wo_c", [JE, 128, JE * 128]),
        'wq_s': din("wq_s", [JE, 128, JE * 128]),
        'wk_s': din("wk_s", [JE, 128, JE * 128]),
        'wo_s': din("wo_s", [JE, 128, JE * 128]),
        'w1': din("w1", [JF, 128, JE * 128]),
        'w2': din("w2", [JF, 128, JE * 128]),
        # V weights: [jin, 128, E] (row blocks of original)
        'wv_c': din("wv_c", [JC, 128, E]),
        'wv_s': din("wv_s", [JE, 128, E]),
    }
    expb_c_d = din("expb_c", [H, L, S])
    expb_s_d = din("expb_s", [H, S, S])
    VIDX = {}
    _off = 0
    for nm, ln in [('cn_g', JE), ('cn_b', JE), ('sn_g', JE), ('sn_b', JE),
                   ('fn_g', JE), ('fn_b', JE), ('bq_c', JE), ('bk_c', JE),
                   ('bo_c', JE), ('bq_s', JE), ('bk_s', JE), ('bo_s', JE),
                   ('b1', JF), ('b2', JE)]:
        VIDX[nm] = _off
        _off += ln
    NV = _off
    vecs_d = din("vecs", [128, NV], F32)
    yT_d = nc.dram_tensor("yT", [E, S], F32, kind="ExternalOutput").ap()

    with tile.TileContext(nc) as tc:
        with tc.tile_pool(name="const", bufs=1) as cpool, \
             tc.tile_pool(name="acts", bufs=1) as acts, \
             tc.tile_pool(name="wst", bufs=8) as wst, \
             tc.tile_pool(name="tr", bufs=2) as tr, \
             tc.tile_pool(name="pe", bufs=4) as pepool, \
             tc.tile_pool(name="eb", bufs=4) as ebpool, \
             tc.tile_pool(name="ps", bufs=1, space="PSUM") as ps:

            def T(pool, shape, dtype, tag, bufs=1):
                return pool.tile(shape, dtype, tag=tag, name=tag, bufs=bufs)

            ones = T(cpool, [128, 128], FR, "ones")
            ones16 = T(cpool, [128, 128], F16, "ones16")
            ones_f = T(cpool, [128, 128], F32, "ones_f")
            epsc = T(cpool, [128, 1], F32, "epsc")
            nc.vector.memset(epsc[:], EPS)
            nc.vector.memset(ones_f[:], 1.0)
            nc.vector.tensor_copy(ones[:], ones_f[:])
            nc.vector.tensor_copy(ones16[:], ones_f[:])
            vecs = T(cpool, [128, NV], F32, "vecs")
            nc.sync.dma_start(vecs[:], vecs_d[:])

            def vap(nm, j):
                return vecs[:, VIDX[nm] + j:VIDX[nm] + j + 1]

            # persistent activation tiles
            rA = [T(acts, [128, S], FR, f"rA{j}") for j in range(JE)]
            rB = [T(acts, [128, S], FR, f"rB{j}") for j in range(JE)]
            lnT = [T(acts, [128, S], F16, f"ln{j}") for j in range(JE)]
            KT = [T(acts, [128, S], F16, f"KT{j}") for j in range(JE)]
            # V tiles: per head 128 cols = [64 ones | 64 values]
            Vx = [T(acts, [128, H * 128], F16, f"V{t}") for t in range(KVS)]
            QT = [T(acts, [128, QCH], F16, f"QT{j}") for j in range(JE)]
            QTb = [T(acts, [128, QCH], F16, f"QU{j}") for j in range(JE)]
            AT = [T(acts, [128, QCH], F16, f"AT{j}") for j in range(JE)]
            ctxT = [T(acts, [128, L], F16, f"cx{j}") for j in range(JC)]

            def vx_strided(t, head0, nh, ones_cols):
                vp = Vx[t][:]
                pstride = vp.ap[0][0]
                off = vp.offset + head0 * 128 + (0 if ones_cols else 64)
                return BassAP(vp.tensor, off,
                              [[pstride, 128], [128, nh], [1, 64]])

            for t in range(KVS):
                nc.vector.memset(vx_strided(t, 0, H, True), 1.0)

            for j in range(JC):
                nc.sync.dma_start(ctxT[j][:], ctxT_d[j * 128:(j + 1) * 128, :])

            def load_xT():
                # emitted after cross-K/V weight DMAs: ring order = emission
                # order, so K/V weights land first and the PE starts at once
                for half in range(2):
                    cs_ = slice(half * QCH, (half + 1) * QCH)
                    for j in range(JE):
                        ring = nc.sync if j % 2 == 0 else nc.scalar
                        ring.dma_start(
                            rA[j][:, cs_],
                            xT_d[j * 128:(j + 1) * 128, cs_].bitcast(FR))

            # PSUM: 3x double-bank "scp" tiles (scores/general) + 2 single
            # "pv" banks = 8 banks total.
            def psum_tile(tag, n=QCH):
                return ps.tile([128, n], F32, tag=tag, name=tag)

            def scp_tile(k):
                return psum_tile(f"scp{k % 3}", 2 * QCH)

            _rot = {'i': 0, 'cur': None}

            def rot_ps(n=QCH):
                i = _rot['i']
                _rot['i'] += 1
                if i % 2 == 0:
                    _rot['cur'] = scp_tile(i // 2)
                return _rot['cur'][:, (i % 2) * QCH:(i % 2) * QCH + n]

            # ---------------- layer norm (transposed layout) ----------------
            def ln_phase(src, dst, gname, bname, affine, only_qc=None):
                inv = 1.0 / float(E)
                for qc in range(NQ):
                    if only_qc is not None and qc != only_qc:
                        continue
                    qs = slice(qc * QCH, (qc + 1) * QCH)
                    sqs = []
                    for j in range(JE):
                        sq = T(tr, [128, QCH], F16, "sq", bufs=6)
                        nc.scalar.activation(sq[:], src[j][:, qs], AF.Square)
                        sqs.append(sq)
                    s12 = scp_tile(0)
                    s1 = s12[:, 0:QCH]
                    s2 = s12[:, QCH:2 * QCH]
                    for j in range(JE):
                        nc.tensor.matmul(s1, ones[:, 0:128], src[j][:, qs],
                                         start=(j == 0), stop=(j == JE - 1))
                    for j in range(JE):
                        nc.tensor.matmul(s2, ones16[:, 0:128], sqs[j][:],
                                         start=(j == 0), stop=(j == JE - 1))
                    t1 = T(tr, [128, QCH], F32, "t1m", bufs=1)
                    nc.scalar.activation(t1[:], s1[:], AF.Square, scale=inv)
                    var = T(tr, [128, QCH], F32, "var", bufs=1)
                    nc.vector.scalar_tensor_tensor(var[:], s2[:], inv, t1[:],
                                                   op0=OP.mult, op1=OP.subtract)
                    nc.scalar.activation(var[:], var[:], AF.Sqrt,
                                         bias=epsc[:, 0:1])
                    rstd = T(tr, [128, QCH], F32, "rstd", bufs=1)
                    nc.vector.reciprocal_approx_fast(rstd[:], var[:])
                    m1r = T(tr, [128, QCH], F32, "m1r", bufs=1)
                    nc.vector.scalar_tensor_tensor(m1r[:], s1[:], inv, rstd[:],
                                                   op0=OP.mult, op1=OP.mult)
                    for j in range(JE):
                        tmp = T(tr, [128, QCH], F32, "lntmp", bufs=2)
                        nc.vector.tensor_tensor(tmp[:], src[j][:, qs], rstd[:],
                                                op=OP.mult)
                        if affine:
                            tmp2 = T(tr, [128, QCH], F32, "lntmp2", bufs=2)
                            nc.vector.tensor_tensor(tmp2[:], tmp[:], m1r[:],
                                                    op=OP.subtract)
                            nc.vector.tensor_scalar(dst[j][:, qs], tmp2[:],
                                                    vap(gname, j), vap(bname, j),
                                                    op0=OP.mult, op1=OP.add)
                        else:
                            nc.vector.tensor_tensor(dst[j][:, qs], tmp[:],
                                                    m1r[:], op=OP.subtract)

            # -------- projection from pre-tiled weights --------
            def wload(wd, of, jin, ring=None):
                wt = T(wst, [128, JE * 128], F16, "wg", bufs=6)
                (ring or nc.sync).dma_start(wt[:, 0:jin * 128], wd[of])
                return wt

            _cpn = {'i': 0}

            def wchain(wt, jin, src_getter, out_ap, n, bias_ap, pt=None):
                if pt is None:
                    pt = rot_ps(n)
                for j in range(jin):
                    nc.tensor.matmul(pt, wt[:, j * 128:(j + 1) * 128],
                                     src_getter(j),
                                     start=(j == 0), stop=(j == jin - 1))
                if bias_ap is not None:
                    nc.vector.tensor_scalar(out_ap, pt, bias_ap, None,
                                            op0=OP.add)
                elif _cpn['i'] % 2 == 0:
                    _cpn['i'] += 1
                    nc.scalar.copy(out_ap, pt)
                else:
                    _cpn['i'] += 1
                    nc.vector.tensor_copy(out_ap, pt)

            def qproj_closures(prefix, wq, lnt, qs, qt_set, pslot=None):
                outs = []
                for of in range(JE):
                    def one(of=of):
                        wt = wload(wq, of, JE)
                        wchain(wt, JE, lambda j: lnt[j][:, qs],
                               qt_set[of][:], QCH,
                               vap(f'bq_{prefix}', of)
                               if flags[f'bq_{prefix}'] else None,
                               pt=pslot() if pslot else None)
                    outs.append(one)
                return outs

            _frot = {'i': 0, 'cur': None}

            def fill_ps(n=QCH):
                i = _frot['i']
                _frot['i'] += 1
                if i % 2 == 0:
                    _frot['cur'] = psum_tile("scp2", 2 * QCH)
                return _frot['cur'][:, (i % 2) * QCH:(i % 2) * QCH + n]

            # ---------------- K/V projection emission ----------------
            def emit_kv(prefix, kv_src, wk, wv, jin_kv, kv_len):
                nkv = kv_len // 128

                _kps = {'i': 0}

                def emit_k(of):
                    wt = wload(wk, of, jin_kv)
                    for ks in range(0, kv_len, QCH):
                        n = min(QCH, kv_len - ks)
                        kp = psum_tile(f"pv{_kps['i'] % 2}")
                        _kps['i'] += 1
                        wchain(wt, jin_kv,
                               lambda j: kv_src[j][:, ks:ks + n],
                               KT[of][:, ks:ks + n], n,
                               vap(f'bk_{prefix}', of)
                               if flags[f'bk_{prefix}'] else None,
                               pt=kp[:, 0:n])

                def emit_vgroup(os_, tg):
                    n = min(QCH, E - os_)
                    tcnt = min(4, nkv - tg)
                    vts = [scp_tile(1), scp_tile(2)]
                    vps = [vts[i // 2][:, (i % 2) * QCH:(i % 2) * QCH + n]
                           for i in range(tcnt)]
                    for j in range(jin_kv):
                        wt = T(wst, [128, QCH], F16, "wv", bufs=3)
                        nc.sync.dma_start(wt[:, 0:n], wv[j, :, os_:os_ + n])
                        for i in range(tcnt):
                            nc.tensor.matmul(
                                vps[i][:, 0:n],
                                kv_src[j][:, (tg + i) * 128:(tg + i + 1) * 128],
                                wt[:, 0:n], start=(j == 0),
                                stop=(j == jin_kv - 1))
                    for i in range(tcnt):
                        dst = vx_strided(tg + i, os_ // 64, n // 64, False)
                        src = vps[i][:, 0:n].rearrange("p (h d) -> p h d", d=64)
                        nc.scalar.copy(dst, src)

                vgroups = [(os_, tg) for os_ in range(0, E, QCH)
                           for tg in range(0, nkv, 4)]
                for i in range(max(JE, len(vgroups))):
                    if i < JE:
                        emit_k(i)
                    if i < len(vgroups):
                        emit_vgroup(*vgroups[i])

            # ---------------- attention (shared cross/self) ----------------
            def attention(prefix, lnt, kv_src, expb_d, res_in,
                          res_out, wq, wk, wv, wo, jin_kv, kv_len,
                          post_qc=None, kv_done=False, qt_sets=None,
                          emit_q=(True, True), fillers=None, rot_mod=3):
                if not kv_done:
                    emit_kv(prefix, kv_src, wk, wv, jin_kv, kv_len)
                if qt_sets is None:
                    qt_sets = [QT, QT]

                for qc in range(NQ):
                    PHASES.append((f'{prefix}:qc{qc}',
                                   int(__import__('re').findall(
                                       r'\d+', nc.get_next_instruction_name())[0])))
                    qs = slice(qc * QCH, (qc + 1) * QCH)
                    qt = qt_sets[qc]
                    # Q^T for this q-chunk (scale folded into wq on host)
                    if emit_q[qc]:
                        for fn_ in qproj_closures(prefix, wq, lnt, qs, qt):
                            fn_()
                    fq = list(fillers[qc]) if fillers else []
                    nkt = kv_len // 128
                    npair = nkt // 2
                    seq = [(h, kp) for h in range(H) for kp in range(npair)]
                    _sr = {'i': 0}
                    state = {}

                    def load_eb(h):
                        ebts = []
                        for kp in range(npair):
                            ebt = T(ebpool, [128, 2 * QCH], F16, "eb", bufs=8)
                            ring = nc.gpsimd if (h % 2 == 0) else nc.scalar
                            ring.dma_start(
                                ebt[:].rearrange("p (t c) -> p t c", t=2),
                                expb_d[h, kp * 256:(kp + 1) * 256, qs]
                                .rearrange("(t p) c -> p t c", p=128))
                            ebts.append(ebt)
                        state.setdefault(h, {'tiles': []})['ebts'] = ebts

                    load_eb(0)

                    def s_stage(i):
                        h, kp = seq[i]
                        st = state.setdefault(h, {'tiles': []})
                        if kp == 0 and h + 1 < H:
                            load_eb(h + 1)
                        th, ph = (h * D) // 128, (h * D) % 128
                        sc = scp_tile(_sr['i'] % rot_mod)
                        _sr['i'] += 1
                        for half in range(2):
                            kvt = 2 * kp + half
                            chain(nc.tensor.matmul(
                                sc[:, half * QCH:(half + 1) * QCH],
                                KT[th][ph:ph + D, kvt * 128:(kvt + 1) * 128],
                                qt[th][ph:ph + D, :], start=True, stop=True))
                        pe = T(pepool, [128, 2 * QCH], F16, "pe", bufs=6)
                        nc.scalar.activation(pe[:], sc[:], AF.Exp)
                        nc.vector.tensor_tensor(pe[:], pe[:],
                                                st['ebts'][kp][:], op=OP.mult)
                        st['tiles'].append(pe)

                    def pv_stage(i):
                        h, kp = seq[i]
                        st = state[h]
                        th, ph = (h * D) // 128, (h * D) % 128
                        if kp == 0:
                            st['pv'] = psum_tile(f"pv{h % 2}")
                        pe = st['tiles'][kp]
                        for half in range(2):
                            kvt = 2 * kp + half
                            chain(nc.tensor.matmul(
                                st['pv'][:], Vx[kvt][:, h * 128:(h + 1) * 128],
                                pe[:, half * QCH:(half + 1) * QCH],
                                start=(kvt == 0), stop=(kvt == nkt - 1)))
                        if kp == npair - 1:
                            pv = st['pv']

                            def fin(pv=pv, th=th, ph=ph):
                                # psum rows 0:64 = kv-sums, 64:128 = P@V
                                rec = T(tr, [64, QCH], F32, "rec", bufs=2)
                                nc.vector.reciprocal_approx_fast(
                                    rec[:], pv[0:64, :])
                                nc.vector.tensor_tensor(AT[th][ph:ph + D, :],
                                                        pv[64:128, :],
                                                        rec[:], op=OP.mult)
                            deferred.append(fin)
                            del state[h]

                    _pe_chain = {'prev': None}

                    def chain(bi):
                        if _pe_chain['prev'] is not None:
                            add_dep_helper(bi.ins, _pe_chain['prev'].ins,
                                           sync=False, reason="pe-order")
                        _pe_chain['prev'] = bi

                    deferred = []
                    BLK = 3
                    blocks = [list(range(i, min(i + BLK, len(seq))))
                              for i in range(0, len(seq), BLK)]
                    for j in range(len(blocks) + 1):
                        if j < len(blocks):
                            for i in blocks[j]:
                                s_stage(i)
                        if fq and j >= 1:
                            fq.pop(0)()
                        while len(deferred) > 1:
                            deferred.pop(0)()
                        if j >= 1:
                            for i in blocks[j - 1]:
                                pv_stage(i)
                    while deferred:
                        deferred.pop(0)()
                    for fn_ in fq:
                        fn_()
                    # out-projection + residual
                    for of in range(JE):
                        wt = wload(wo, of, JE)
                        pt = rot_ps()
                        for j in range(JE):
                            nc.tensor.matmul(pt, wt[:, j * 128:(j + 1) * 128],
                                             AT[j][:],
                                             start=(j == 0), stop=(j == JE - 1))
                        if flags[f'bo_{prefix}']:
                            nc.vector.scalar_tensor_tensor(
                                res_out[of][:, qs], pt, vap(f'bo_{prefix}', of),
                                res_in[of][:, qs], op0=OP.add, op1=OP.add)
                        else:
                            nc.vector.tensor_tensor(res_out[of][:, qs], pt,
                                                    res_in[of][:, qs], op=OP.add)
                    if post_qc is not None:
                        post_qc(qc)

            # ================= the layer =================
            import re as _re

            def _mark(lbl):
                n = int(_re.findall(r'\d+', nc.get_next_instruction_name())[0])
                PHASES.append((lbl, n))

            _mark('ckv')
            # cross K/V only need ctxT: emit first to overlap xT DMA
            emit_kv('c', ctxT, w_d['wk_c'], w_d['wv_c'], JC, L)
            load_xT()
            _mark('ln1')
            ln_phase(rA, lnT, 'cn_g', 'cn_b', flags['cn'])
            _mark('cross')
            qs1 = slice(QCH, 2 * QCH)
            qs0 = slice(0, QCH)
            fill_q0 = qproj_closures('c', w_d['wq_c'], lnT, qs1, QTb,
                                     pslot=fill_ps)
            fill_q1 = qproj_closures('s', w_d['wq_s'], lnT, qs0, QT,
                                     pslot=fill_ps)
            attention('c', lnT, ctxT, expb_c_d, rA, rB,
                      w_d['wq_c'], w_d['wk_c'], w_d['wv_c'], w_d['wo_c'],
                      JC, L, kv_done=True, rot_mod=2,
                      qt_sets=[QT, QTb], emit_q=(True, False),
                      fillers=[fill_q0, fill_q1],
                      post_qc=lambda qc: ln_phase(rB, lnT, 'sn_g', 'sn_b',
                                                  flags['sn'], only_qc=qc))
            _mark('self')
            attention('s', lnT, lnT, expb_s_d, rB, rA,
                      w_d['wq_s'], w_d['wk_s'], w_d['wv_s'], w_d['wo_s'],
                      JE, S, qt_sets=[QT, QTb], emit_q=(False, True),
                      post_qc=lambda qc: ln_phase(rA, lnT, 'fn_g', 'fn_b',
                                                  flags['fn'], only_qc=qc))
            _mark('ffn')

            # ================= FFN =================
            for qc in range(NQ):
                qs = slice(qc * QCH, (qc + 1) * QCH)
                ytiles = [scp_tile(k) for k in range(3)]
                ypt = [ytiles[k // 2][:, (k % 2) * QCH:(k % 2 + 1) * QCH]
                       for k in range(JE)]

                def emit_f1(of):
                    wt = T(wst, [128, JE * 128], F16, "w1g", bufs=4)
                    nc.sync.dma_start(wt[:], w_d['w1'][of])
                    f1 = psum_tile(f"pv{of % 2}")
                    for j in range(JE):
                        nc.tensor.matmul(f1[:], wt[:, j * 128:(j + 1) * 128],
                                         lnT[j][:, qs],
                                         start=(j == 0), stop=(j == JE - 1))
                    g = T(tr, [128, QCH], F16, "gelu", bufs=3)
                    nc.scalar.activation(g[:], f1[:], AF.Gelu_apprx_tanh,
                                         bias=vap('b1', of) if flags['b1'] else 0.0)
                    return g

                def load_w2(of):
                    w2t = T(wst, [128, JE * 128], F16, "w2g", bufs=4)
                    nc.gpsimd.dma_start(w2t[:], w_d['w2'][of])
                    return w2t

                gprev = emit_f1(0)
                w2prev = load_w2(0)
                for of in range(JF):
                    gnext = emit_f1(of + 1) if of + 1 < JF else None
                    w2next = load_w2(of + 1) if of + 1 < JF else None
                    for of2 in range(JE):
                        nc.tensor.matmul(ypt[of2],
                                         w2prev[:, of2 * 128:(of2 + 1) * 128],
                                         gprev[:],
                                         start=(of == 0), stop=(of == JF - 1))
                    gprev = gnext
                    w2prev = w2next
                for of2 in range(JE):
                    yo = T(tr, [128, QCH], F32, "yout", bufs=6)
                    if flags['b2']:
                        nc.vector.tensor_scalar(yo[:], ypt[of2], vap('b2', of2),
                                                None, op0=OP.add)
                    else:
                        nc.vector.tensor_copy(yo[:], ypt[of2])
                    ring = [nc.sync, nc.scalar][of2 % 2]
                    ring.dma_start(yT_d[of2 * 128:(of2 + 1) * 128, qs], yo[:])

    nc.compile()
    return nc


def kernel(**inputs):
    inp = {k: np.asarray(v, dtype=np.float32) for k, v in inputs.items()}
    triv1 = lambda v: bool(np.all(v == 1.0))
    triv0 = lambda v: bool(np.all(v == 0.0))
    flags = {
        'cn': not (triv1(inp['cn_g']) and triv0(inp['cn_b'])),
        'sn': not (triv1(inp['sn_g']) and triv0(inp['sn_b'])),
        'fn': not (triv1(inp['fn_g']) and triv0(inp['fn_b'])),
        'bq_c': not triv0(inp['bq_c']), 'bk_c': not triv0(inp['bk_c']),
        'bo_c': not triv0(inp['bo_c']), 'bq_s': not triv0(inp['bq_s']),
        'bk_s': not triv0(inp['bk_s']), 'bo_s': not triv0(inp['bo_s']),
        'b1': not triv0(inp['b1']), 'b2': not triv0(inp['b2']),
    }
    assert triv0(inp['bv_c']) and triv0(inp['bv_s']), \
        "nonzero V bias not supported by this build"

    key = tuple(sorted(flags.items()))
    if key not in _BUILT:
        _BUILT[key] = _build(flags)
    nc = _BUILT[key]

    from concourse.bass_utils import run_bass_kernel_spmd

    f16 = np.float16
    scale = 1.0 / np.sqrt(np.float32(D))

    def tile_w(W, jin, ofn):
        return np.ascontiguousarray(
            W.reshape(jin, 128, ofn, 128).transpose(2, 1, 0, 3)
            .reshape(ofn, 128, jin * 128).astype(f16))

    def tile_v(W, jin):
        return np.ascontiguousarray(W.reshape(jin, 128, E).astype(f16))

    com = {
        'wq_c': tile_w(inp['wq_c'] * scale, JE, JE),
        'wk_c': tile_w(inp['wk_c'], JC, JE),
        'wv_c': tile_v(inp['wv_c'], JC),
        'wo_c': tile_w(inp['wo_c'], JE, JE),
        'wq_s': tile_w(inp['wq_s'] * scale, JE, JE),
        'wk_s': tile_w(inp['wk_s'], JE, JE),
        'wv_s': tile_v(inp['wv_s'], JE),
        'wo_s': tile_w(inp['wo_s'], JE, JE),
        'w1': tile_w(inp['w1'], JE, JF),
        'w2': np.ascontiguousarray(inp['w2'].reshape(JF, 128, E).astype(f16)),
        'expb_c': np.ascontiguousarray(
            np.exp(inp['bias_c'].transpose(0, 2, 1)).astype(f16)),
        'expb_s': np.ascontiguousarray(
            np.exp(inp['bias_s'].transpose(0, 2, 1)).astype(f16)),
    }
    chunks = []
    for nm in ['cn_g', 'cn_b', 'sn_g', 'sn_b', 'fn_g', 'fn_b']:
        chunks.append(inp[nm].reshape(-1, 128))
    chunks.append((inp['bq_c'] * scale).reshape(-1, 128))
    for nm in ['bk_c', 'bo_c']:
        chunks.append(inp[nm].reshape(-1, 128))
    chunks.append((inp['bq_s'] * scale).reshape(-1, 128))
    for nm in ['bk_s', 'bo_s', 'b1', 'b2']:
        chunks.append(inp[nm].reshape(-1, 128))
    com['vecs'] = np.ascontiguousarray(np.concatenate(chunks, 0).T)

    in_maps = []
    for b in range(B):
        m = dict(com)
        m['xT'] = np.ascontiguousarray(inp['hidden_state'][b].T)
        m['ctxT'] = np.ascontiguousarray(inp['context'][b].T.astype(f16))
        in_maps.append(m)

    res = run_bass_kernel_spmd(nc, in_maps, core_ids=list(range(NCORES)),
                               trace=TRACE)
    LAST['res'] = res
    y = np.stack([res.results[c]['yT'].T for c in range(B)])
    return np.ascontiguousarray(y.astype(np.float32))


# revision 19
# speedup vs baseline: 1.0520x; 1.0074x over previous
"""Trainium2 Bass kernel for nn_BasicTransformerLayer (dense transformer layer).

Strategy v2:
- Data-parallel over batch: B=8, one batch element per NeuronCore, no
  collectives.
- Activations transposed [features, tokens]; residual stream in fp32r,
  everything on matmul paths in float16 (weights pre-tiled on host into
  contiguous per-output-tile blocks for max DMA efficiency).
- Softmax: exp(scores) on scalar engine (f16), multiplicative exp(bias)
  (host-precomputed, f16) on vector; the kv-sum (softmax denominator) is
  folded into the P@V matmul via V tiles laid out [64 ones | 64 values]
  per head: psum rows 0:64 = sums, 64:128 = P@V.
- DMA spread across sync/scalar/gpsimd rings; output written per-psum-bank
  with rotating buffers to avoid a serialized tail.
"""
import sys

sys.path.insert(0, '/opt/trn_rl_repo')

import numpy as np

E, C, H, D, FF = 768, 512, 12, 64, 3072
B, S, L = 8, 1024, 256
EPS = 1e-5
NCORES = 8
QCH = 512                  # q-chunk (matmul moving free dim)
NQ = S // QCH              # 2
JE = E // 128              # 6 feature tiles
JC = C // 128              # 4 cross-feature tiles
JF = FF // 128             # 24 ffn tiles
KVS = S // 128             # 8 self kv tiles
KVC = L // 128             # 2 cross kv tiles

_BUILT = {}
TRACE = False
LAST = {}
PHASES = []


def _build(flags):
    import concourse.bacc as bacc
    import concourse.mybir as mybir
    import concourse.tile as tile
    from concourse.tile import add_dep_helper
    from concourse.bass import AP as BassAP

    FR = mybir.dt.float32r
    F32 = mybir.dt.float32
    F16 = mybir.dt.float16
    AF = mybir.ActivationFunctionType
    OP = mybir.AluOpType

    nc = bacc.Bacc("TRN2", target_bir_lowering=False, debug=False,
                   enable_asserts=True, num_devices=NCORES)

    def din(name, shape, dt=F16):
        return nc.dram_tensor(name, shape, dt, kind="ExternalInput").ap()

    xT_d = din("xT", [E, S], F32)
    ctxT_d = din("ctxT", [C, L])
    # pre-tiled weights: [ofn, 128, jin*128] f16 (contiguous per of)
    w_d = {
        'wq_c': din("wq_c", [JE, 128, JE * 128]),
        'wk_c': din("wk_c", [JE, 128, JC * 128]),
        'wo_c': din("wo_c", [JE, 128, JE * 128]),
        'wq_s': din("wq_s", [JE, 128, JE * 128]),
        'wk_s': din("wk_s", [JE, 128, JE * 128]),
        'wo_s': din("wo_s", [JE, 128, JE * 128]),
        'w1': din("w1", [JF, 128, JE * 128]),
        'w2': din("w2", [JF, 128, JE * 128]),
        # V weights: [jin, 128, E] (row blocks of original)
        'wv_c': din("wv_c", [JC, 128, E]),
        'wv_s': din("wv_s", [JE, 128, E]),
    }
    expb_c_d = din("expb_c", [H, L, S])
    expb_s_d = din("expb_s", [H, S, S])
    VIDX = {}
    _off = 0
    for nm, ln in [('cn_g', JE), ('cn_b', JE), ('sn_g', JE), ('sn_b', JE),
                   ('fn_g', JE), ('fn_b', JE), ('bq_c', JE), ('bk_c', JE),
                   ('bo_c', JE), ('bq_s', JE), ('bk_s', JE), ('bo_s', JE),
                   ('b1', JF), ('b2', JE)]:
        VIDX[nm] = _off
        _off += ln
    NV = _off
    vecs_d = din("vecs", [128, NV], F32)
    yT_d = nc.dram_tensor("yT", [E, S], F32, kind="ExternalOutput").ap()

    with tile.TileContext(nc) as tc:
        with tc.tile_pool(name="const", bufs=1) as cpool, \
             tc.tile_pool(name="acts", bufs=1) as acts, \
             tc.tile_pool(name="wst", bufs=8) as wst, \
             tc.tile_pool(name="tr", bufs=2) as tr, \
             tc.tile_pool(name="pe", bufs=4) as pepool, \
             tc.tile_pool(name="eb", bufs=4) as ebpool, \
             tc.tile_pool(name="ps", bufs=1, space="PSUM") as ps:

            def T(pool, shape, dtype, tag, bufs=1):
                return pool.tile(shape, dtype, tag=tag, name=tag, bufs=bufs)

            ones = T(cpool, [128, 128], FR, "ones")
            ones16 = T(cpool, [128, 128], F16, "ones16")
            ones_f = T(cpool, [128, 128], F32, "ones_f")
            epsc = T(cpool, [128, 1], F32, "epsc")
            nc.vector.memset(epsc[:], EPS)
            nc.vector.memset(ones_f[:], 1.0)
            nc.vector.tensor_copy(ones[:], ones_f[:])
            nc.vector.tensor_copy(ones16[:], ones_f[:])
            vecs = T(cpool, [128, NV], F32, "vecs")
            nc.sync.dma_start(vecs[:], vecs_d[:])

            def vap(nm, j):
                return vecs[:, VIDX[nm] + j:VIDX[nm] + j + 1]

            # persistent activation tiles
            rA = [T(acts, [128, S], FR, f"rA{j}") for j in range(JE)]
            rB = [T(acts, [128, S], FR, f"rB{j}") for j in range(JE)]
            lnT = [T(acts, [128, S], F16, f"ln{j}") for j in range(JE)]
            KT = [T(acts, [128, S], F16, f"KT{j}") for j in range(JE)]
            # V tiles: per head 128 cols = [64 ones | 64 values]
            Vx = [T(acts, [128, H * 128], F16, f"V{t}") for t in range(KVS)]
            QT = [T(acts, [128, QCH], F16, f"QT{j}") for j in range(JE)]
            QTb = [T(acts, [128, QCH], F16, f"QU{j}") for j in range(JE)]
            AT = [T(acts, [128, QCH], F16, f"AT{j}") for j in range(JE)]
            ctxT = [T(acts, [128, L], F16, f"cx{j}") for j in range(JC)]

            def vx_strided(t, head0, nh, ones_cols):
                vp = Vx[t][:]
                pstride = vp.ap[0][0]
                off = vp.offset + head0 * 128 + (0 if ones_cols else 64)
                return BassAP(vp.tensor, off,
                              [[pstride, 128], [128, nh], [1, 64]])

            for t in range(KVS):
                nc.vector.memset(vx_strided(t, 0, H, True), 1.0)

            for j in range(JC):
                nc.sync.dma_start(ctxT[j][:], ctxT_d[j * 128:(j + 1) * 128, :])

            def load_xT():
                # emitted after cross-K/V weight DMAs: ring order = emission
                # order, so K/V weights land first and the PE starts at once
                for half in range(2):
                    cs_ = slice(half * QCH, (half + 1) * QCH)
                    for j in range(JE):
                        ring = nc.sync if j % 2 == 0 else nc.scalar
                        ring.dma_start(
                            rA[j][:, cs_],
                            xT_d[j * 128:(j + 1) * 128, cs_].bitcast(FR))

            # PSUM: 3x double-bank "scp" tiles (scores/general) + 2 single
            # "pv" banks = 8 banks total.
            def psum_tile(tag, n=QCH):
                return ps.tile([128, n], F32, tag=tag, name=tag)

            def scp_tile(k):
                return psum_tile(f"scp{k % 3}", 2 * QCH)

            _rot = {'i': 0, 'cur': None}

            def rot_ps(n=QCH):
                i = _rot['i']
                _rot['i'] += 1
                if i % 2 == 0:
                    _rot['cur'] = scp_tile(i // 2)
                return _rot['cur'][:, (i % 2) * QCH:(i % 2) * QCH + n]

            # ---------------- layer norm (transposed layout) ----------------
            def ln_phase(src, dst, gname, bname, affine, only_qc=None):
                inv = 1.0 / float(E)
                for qc in range(NQ):
                    if only_qc is not None and qc != only_qc:
                        continue
                    qs = slice(qc * QCH, (qc + 1) * QCH)
                    sqs = []
                    for j in range(JE):
                        sq = T(tr, [128, QCH], F16, "sq", bufs=6)
                        nc.scalar.activation(sq[:], src[j][:, qs], AF.Square)
                        sqs.append(sq)
                    s12 = scp_tile(0)
                    s1 = s12[:, 0:QCH]
                    s2 = s12[:, QCH:2 * QCH]
                    for j in range(JE):
                        nc.tensor.matmul(s1, ones[:, 0:128], src[j][:, qs],
                                         start=(j == 0), stop=(j == JE - 1))
                    for j in range(JE):
                        nc.tensor.matmul(s2, ones16[:, 0:128], sqs[j][:],
                                         start=(j == 0), stop=(j == JE - 1))
                    t1 = T(tr, [128, QCH], F32, "t1m", bufs=1)
                    nc.scalar.activation(t1[:], s1[:], AF.Square, scale=inv)
                    var = T(tr, [128, QCH], F32, "var", bufs=1)
                    nc.vector.scalar_tensor_tensor(var[:], s2[:], inv, t1[:],
                                                   op0=OP.mult, op1=OP.subtract)
                    nc.scalar.activation(var[:], var[:], AF.Sqrt,
                                         bias=epsc[:, 0:1])
                    rstd = T(tr, [128, QCH], F32, "rstd", bufs=1)
                    nc.vector.reciprocal_approx_fast(rstd[:], var[:])
                    m1r = T(tr, [128, QCH], F32, "m1r", bufs=1)
                    nc.vector.scalar_tensor_tensor(m1r[:], s1[:], inv, rstd[:],
                                                   op0=OP.mult, op1=OP.mult)
                    for j in range(JE):
                        tmp = T(tr, [128, QCH], F32, "lntmp", bufs=2)
                        nc.vector.tensor_tensor(tmp[:], src[j][:, qs], rstd[:],
                                                op=OP.mult)
                        if affine:
                            tmp2 = T(tr, [128, QCH], F32, "lntmp2", bufs=2)
                            nc.vector.tensor_tensor(tmp2[:], tmp[:], m1r[:],
                                                    op=OP.subtract)
                            nc.vector.tensor_scalar(dst[j][:, qs], tmp2[:],
                                                    vap(gname, j), vap(bname, j),
                                                    op0=OP.mult, op1=OP.add)
                        else:
                            nc.vector.tensor_tensor(dst[j][:, qs], tmp[:],
                                                    m1r[:], op=OP.subtract)

            # -------- projection from pre-tiled weights --------
            def wload(wd, of, jin, ring=None):
                wt = T(wst, [128, JE * 128], F16, "wg", bufs=6)
                (ring or nc.sync).dma_start(wt[:, 0:jin * 128], wd[of])
                return wt

            _cpn = {'i': 0}

            def wchain(wt, jin, src_getter, out_ap, n, bias_ap, pt=None):
                if pt is None:
                    pt = rot_ps(n)
                for j in range(jin):
                    nc.tensor.matmul(pt, wt[:, j * 128:(j + 1) * 128],
                                     src_getter(j),
                                     start=(j == 0), stop=(j == jin - 1))
                if bias_ap is not None:
                    nc.vector.tensor_scalar(out_ap, pt, bias_ap, None,
                                            op0=OP.add)
                elif _cpn['i'] % 2 == 0:
                    _cpn['i'] += 1
                    nc.scalar.copy(out_ap, pt)
                else:
                    _cpn['i'] += 1
                    nc.vector.tensor_copy(out_ap, pt)

            def qproj_closures(prefix, wq, lnt, qs, qt_set, pslot=None):
                outs = []
                for of in range(JE):
                    def one(of=of):
                        wt = wload(wq, of, JE)
                        wchain(wt, JE, lambda j: lnt[j][:, qs],
                               qt_set[of][:], QCH,
                               vap(f'bq_{prefix}', of)
                               if flags[f'bq_{prefix}'] else None,
                               pt=pslot() if pslot else None)
                    outs.append(one)
                return outs

            _frot = {'i': 0, 'cur': None}

            def fill_ps(n=QCH):
                i = _frot['i']
                _frot['i'] += 1
                if i % 2 == 0:
                    _frot['cur'] = psum_tile("scp2", 2 * QCH)
                return _frot['cur'][:, (i % 2) * QCH:(i % 2) * QCH + n]

            # ---------------- K/V projection emission ----------------
            def emit_kv(prefix, kv_src, wk, wv, jin_kv, kv_len):
                nkv = kv_len // 128

                _kps = {'i': 0}

                def emit_k(of):
                    wt = wload(wk, of, jin_kv)
                    for ks in range(0, kv_len, QCH):
                        n = min(QCH, kv_len - ks)
                        kp = psum_tile(f"pv{_kps['i'] % 2}")
                        _kps['i'] += 1
                        wchain(wt, jin_kv,
                               lambda j: kv_src[j][:, ks:ks + n],
                               KT[of][:, ks:ks + n], n,
                               vap(f'bk_{prefix}', of)
                               if flags[f'bk_{prefix}'] else None,
                               pt=kp[:, 0:n])

                def emit_vgroup(os_, tg):
                    n = min(QCH, E - os_)
                    tcnt = min(4, nkv - tg)
                    vts = [scp_tile(1), scp_tile(2)]
                    vps = [vts[i // 2][:, (i % 2) * QCH:(i % 2) * QCH + n]
                           for i in range(tcnt)]
                    for j in range(jin_kv):
                        wt = T(wst, [128, QCH], F16, "wv", bufs=3)
                        nc.sync.dma_start(wt[:, 0:n], wv[j, :, os_:os_ + n])
                        for i in range(tcnt):
                            nc.tensor.matmul(
                                vps[i][:, 0:n],
                                kv_src[j][:, (tg + i) * 128:(tg + i + 1) * 128],
                                wt[:, 0:n], start=(j == 0),
                                stop=(j == jin_kv - 1))
                    for i in range(tcnt):
                        dst = vx_strided(tg + i, os_ // 64, n // 64, False)
                        src = vps[i][:, 0:n].rearrange("p (h d) -> p h d", d=64)
                        nc.scalar.copy(dst, src)

                vgroups = [(os_, tg) for os_ in range(0, E, QCH)
                           for tg in range(0, nkv, 4)]
                for i in range(max(JE, len(vgroups))):
                    if i < JE:
                        emit_k(i)
                    if i < len(vgroups):
                        emit_vgroup(*vgroups[i])

            # ---------------- attention (shared cross/self) ----------------
            def attention(prefix, lnt, kv_src, expb_d, res_in,
                          res_out, wq, wk, wv, wo, jin_kv, kv_len,
                          post_qc=None, kv_done=False, qt_sets=None,
                          emit_q=(True, True), fillers=None, rot_mod=3):
                if not kv_done:
                    emit_kv(prefix, kv_src, wk, wv, jin_kv, kv_len)
                if qt_sets is None:
                    qt_sets = [QT, QT]

                for qc in range(NQ):
                    PHASES.append((f'{prefix}:qc{qc}',
                                   int(__import__('re').findall(
                                       r'\d+', nc.get_next_instruction_name())[0])))
                    qs = slice(qc * QCH, (qc + 1) * QCH)
                    qt = qt_sets[qc]
                    # Q^T for this q-chunk (scale folded into wq on host)
                    if emit_q[qc]:
                        for fn_ in qproj_closures(prefix, wq, lnt, qs, qt):
                            fn_()
                    fq = list(fillers[qc]) if fillers else []
                    nkt = kv_len // 128
                    npair = nkt // 2
                    seq = [(h, kp) for h in range(H) for kp in range(npair)]
                    _sr = {'i': 0}
                    state = {}

                    def load_eb(h):
                        ebts = []
                        for kp in range(npair):
                            ebt = T(ebpool, [128, 2 * QCH], F16, "eb", bufs=8)
                            ring = nc.gpsimd if (h % 2 == 0) else nc.scalar
                            ring.dma_start(
                                ebt[:].rearrange("p (t c) -> p t c", t=2),
                                expb_d[h, kp * 256:(kp + 1) * 256, qs]
                                .rearrange("(t p) c -> p t c", p=128))
                            ebts.append(ebt)
                        state.setdefault(h, {'tiles': []})['ebts'] = ebts

                    load_eb(0)

                    def s_stage(i):
                        h, kp = seq[i]
                        st = state.setdefault(h, {'tiles': []})
                        if kp == 0 and h + 1 < H:
                            load_eb(h + 1)
                        th, ph = (h * D) // 128, (h * D) % 128
                        sc = scp_tile(_sr['i'] % rot_mod)
                        _sr['i'] += 1
                        for half in range(2):
                            kvt = 2 * kp + half
                            chain(nc.tensor.matmul(
                                sc[:, half * QCH:(half + 1) * QCH],
                                KT[th][ph:ph + D, kvt * 128:(kvt + 1) * 128],
                                qt[th][ph:ph + D, :], start=True, stop=True))
                        pe = T(pepool, [128, 2 * QCH], F16, "pe", bufs=6)
                        nc.scalar.activation(pe[:], sc[:], AF.Exp)
                        nc.vector.tensor_tensor(pe[:], pe[:],
                                                st['ebts'][kp][:], op=OP.mult)
                        st['tiles'].append(pe)

                    def pv_stage(i):
                        h, kp = seq[i]
                        st = state[h]
                        th, ph = (h * D) // 128, (h * D) % 128
                        if kp == 0:
                            st['pv'] = psum_tile(f"pv{h % 2}")
                        pe = st['tiles'][kp]
                        for half in range(2):
                            kvt = 2 * kp + half
                            chain(nc.tensor.matmul(
                                st['pv'][:], Vx[kvt][:, h * 128:(h + 1) * 128],
                                pe[:, half * QCH:(half + 1) * QCH],
                                start=(kvt == 0), stop=(kvt == nkt - 1)))
                        if kp == npair - 1:
                            pv = st['pv']

                            def fin(pv=pv, th=th, ph=ph):
                                # psum rows 0:64 = kv-sums, 64:128 = P@V
                                rec = T(tr, [64, QCH], F32, "rec", bufs=2)
                                nc.vector.reciprocal_approx_fast(
                                    rec[:], pv[0:64, :])
                                nc.vector.tensor_tensor(AT[th][ph:ph + D, :],
                                                        pv[64:128, :],
                                                        rec[:], op=OP.mult)
                            deferred.append(fin)
                            del state[h]

                    _pe_chain = {'prev': None}

                    def chain(bi):
                        if _pe_chain['prev'] is not None:
                            add_dep_helper(bi.ins, _pe_chain['prev'].ins,
                                           sync=False, reason="pe-order")
                        _pe_chain['prev'] = bi

                    deferred = []
                    BLK = 3
                    blocks = [list(range(i, min(i + BLK, len(seq))))
                              for i in range(0, len(seq), BLK)]
                    for j in range(len(blocks) + 1):
                        if j < len(blocks):
                            for i in blocks[j]:
                                s_stage(i)
                        if fq and j >= 1:
                            fq.pop(0)()
                        while len(deferred) > 1:
                            deferred.pop(0)()
                        if j >= 1:
                            for i in blocks[j - 1]:
                                pv_stage(i)
                    while deferred:
                        deferred.pop(0)()
                    for fn_ in fq:
                        fn_()
                    # out-projection + residual
                    for of in range(JE):
                        wt = wload(wo, of, JE)
                        pt = rot_ps()
                        for j in range(JE):
                            nc.tensor.matmul(pt, wt[:, j * 128:(j + 1) * 128],
                                             AT[j][:],
                                             start=(j == 0), stop=(j == JE - 1))
                        if flags[f'bo_{prefix}']:
                            nc.vector.scalar_tensor_tensor(
                                res_out[of][:, qs], pt, vap(f'bo_{prefix}', of),
                                res_in[of][:, qs], op0=OP.add, op1=OP.add)
                        else:
                            nc.vector.tensor_tensor(res_out[of][:, qs], pt,
                                                    res_in[of][:, qs], op=OP.add)
                    if post_qc is not None:
                        post_qc(qc)

            # ================= the layer =================
            import re as _re

            def _mark(lbl):
                n = int(_re.findall(r'\d+', nc.get_next_instruction_name())[0])
                PHASES.append((lbl, n))

            _mark('ckv')
            # cross K/V only need ctxT: emit first to overlap xT DMA
            emit_kv('c', ctxT, w_d['wk_c'], w_d['wv_c'], JC, L)
            load_xT()
            _mark('ln1')
            ln_phase(rA, lnT, 'cn_g', 'cn_b', flags['cn'])
            _mark('cross')
            qs1 = slice(QCH, 2 * QCH)
            qs0 = slice(0, QCH)
            fill_q0 = qproj_closures('c', w_d['wq_c'], lnT, qs1, QTb,
                                     pslot=fill_ps)
            fill_q1 = qproj_closures('s', w_d['wq_s'], lnT, qs0, QT,
                                     pslot=fill_ps)
            attention('c', lnT, ctxT, expb_c_d, rA, rB,
                      w_d['wq_c'], w_d['wk_c'], w_d['wv_c'], w_d['wo_c'],
                      JC, L, kv_done=True, rot_mod=2,
                      qt_sets=[QT, QTb], emit_q=(True, False),
                      fillers=[fill_q0, fill_q1],
                      post_qc=lambda qc: ln_phase(rB, lnT, 'sn_g', 'sn_b',
                                                  flags['sn'], only_qc=qc))
            _mark('self')
            attention('s', lnT, lnT, expb_s_d, rB, rA,
                      w_d['wq_s'], w_d['wk_s'], w_d['wv_s'], w_d['wo_s'],
                      JE, S, qt_sets=[QT, QTb], emit_q=(False, True))
            # fn-ln after self-attn: overlaps FFN weight prefetch/start
            ln_phase(rA, lnT, 'fn_g', 'fn_b', flags['fn'])
            _mark('ffn')

            # ================= FFN =================
            for qc in range(NQ):
                qs = slice(qc * QCH, (qc + 1) * QCH)
                ytiles = [scp_tile(k) for k in range(3)]
                ypt = [ytiles[k // 2][:, (k % 2) * QCH:(k % 2 + 1) * QCH]
                       for k in range(JE)]

                def emit_f1(of):
                    wt = T(wst, [128, JE * 128], F16, "w1g", bufs=4)
                    nc.sync.dma_start(wt[:], w_d['w1'][of])
                    f1 = psum_tile(f"pv{of % 2}")
                    for j in range(JE):
                        nc.tensor.matmul(f1[:], wt[:, j * 128:(j + 1) * 128],
                                         lnT[j][:, qs],
                                         start=(j == 0), stop=(j == JE - 1))
                    g = T(tr, [128, QCH], F16, "gelu", bufs=3)
                    nc.scalar.activation(g[:], f1[:], AF.Gelu_apprx_tanh,
                                         bias=vap('b1', of) if flags['b1'] else 0.0)
                    return g

                def load_w2(of):
                    w2t = T(wst, [128, JE * 128], F16, "w2g", bufs=4)
                    nc.gpsimd.dma_start(w2t[:], w_d['w2'][of])
                    return w2t

                gprev = emit_f1(0)
                w2prev = load_w2(0)
                for of in range(JF):
                    gnext = emit_f1(of + 1) if of + 1 < JF else None
                    w2next = load_w2(of + 1) if of + 1 < JF else None
                    for of2 in range(JE):
                        nc.tensor.matmul(ypt[of2],
                                         w2prev[:, of2 * 128:(of2 + 1) * 128],
                                         gprev[:],
                                         start=(of == 0), stop=(of == JF - 1))
                    gprev = gnext
                    w2prev = w2next
                for of2 in range(JE):
                    yo = T(tr, [128, QCH], F32, "yout", bufs=6)
                    if flags['b2']:
                        nc.vector.tensor_scalar(yo[:], ypt[of2], vap('b2', of2),
                                                None, op0=OP.add)
                    else:
                        nc.vector.tensor_copy(yo[:], ypt[of2])
                    ring = [nc.sync, nc.scalar][of2 % 2]
                    ring.dma_start(yT_d[of2 * 128:(of2 + 1) * 128, qs], yo[:])

    nc.compile()
    return nc


def kernel(**inputs):
    inp = {k: np.asarray(v, dtype=np.float32) for k, v in inputs.items()}
    triv1 = lambda v: bool(np.all(v == 1.0))
    triv0 = lambda v: bool(np.all(v == 0.0))
    flags = {
        'cn': not (triv1(inp['cn_g']) and triv0(inp['cn_b'])),
        'sn': not (triv1(inp['sn_g']) and triv0(inp['sn_b'])),
        'fn': not (triv1(inp['fn_g']) and triv0(inp['fn_b'])),
        'bq_c': not triv0(inp['bq_c']), 'bk_c': not triv0(inp['bk_c']),
        'bo_c': not triv0(inp['bo_c']), 'bq_s': not triv0(inp['bq_s']),
        'bk_s': not triv0(inp['bk_s']), 'bo_s': not triv0(inp['bo_s']),
        'b1': not triv0(inp['b1']), 'b2': not triv0(inp['b2']),
    }
    assert triv0(inp['bv_c']) and triv0(inp['bv_s']), \
        "nonzero V bias not supported by this build"

    key = tuple(sorted(flags.items()))
    if key not in _BUILT:
        _BUILT[key] = _build(flags)
    nc = _BUILT[key]

    from concourse.bass_utils import run_bass_kernel_spmd

    f16 = np.float16
    scale = 1.0 / np.sqrt(np.float32(D))

    def tile_w(W, jin, ofn):
        return np.ascontiguousarray(
            W.reshape(jin, 128, ofn, 128).transpose(2, 1, 0, 3)
            .reshape(ofn, 128, jin * 128).astype(f16))

    def tile_v(W, jin):
        return np.ascontiguousarray(W.reshape(jin, 128, E).astype(f16))

    com = {
        'wq_c': tile_w(inp['wq_c'] * scale, JE, JE),
        'wk_c': tile_w(inp['wk_c'], JC, JE),
        'wv_c': tile_v(inp['wv_c'], JC),
        'wo_c': tile_w(inp['wo_c'], JE, JE),
        'wq_s': tile_w(inp['wq_s'] * scale, JE, JE),
        'wk_s': tile_w(inp['wk_s'], JE, JE),
        'wv_s': tile_v(inp['wv_s'], JE),
        'wo_s': tile_w(inp['wo_s'], JE, JE),
        'w1': tile_w(inp['w1'], JE, JF),
        'w2': np.ascontiguousarray(inp['w2'].reshape(JF, 128, E).astype(f16)),
        'expb_c': np.ascontiguousarray(
            np.exp(inp['bias_c'].transpose(0, 2, 1)).astype(f16)),
        'expb_s': np.ascontiguousarray(
            np.exp(inp['bias_s'].transpose(0, 2, 1)).astype(f16)),
    }
    chunks = []
    for nm in ['cn_g', 'cn_b', 'sn_g', 'sn_b', 'fn_g', 'fn_b']:
        chunks.append(inp[nm].reshape(-1, 128))
    chunks.append((inp['bq_c'] * scale).reshape(-1, 128))
    for nm in ['bk_c', 'bo_c']:
        chunks.append(inp[nm].reshape(-1, 128))
    chunks.append((inp['bq_s'] * scale).reshape(-1, 128))
    for nm in ['bk_s', 'bo_s', 'b1', 'b2']:
        chunks.append(inp[nm].reshape(-1, 128))
    com['vecs'] = np.ascontiguousarray(np.concatenate(chunks, 0).T)

    in_maps = []
    for b in range(B):
        m = dict(com)
        m['xT'] = np.ascontiguousarray(inp['hidden_state'][b].T)
        m['ctxT'] = np.ascontiguousarray(inp['context'][b].T.astype(f16))
        in_maps.append(m)

    res = run_bass_kernel_spmd(nc, in_maps, core_ids=list(range(NCORES)),
                               trace=TRACE)
    LAST['res'] = res
    y = np.stack([res.results[c]['yT'].T for c in range(B)])
    return np.ascontiguousarray(y.astype(np.float32))


# revision 20
# speedup vs baseline: 1.1068x; 1.0521x over previous
"""Trainium2 Bass kernel for nn_BasicTransformerLayer (dense transformer layer).

Strategy v2:
- Data-parallel over batch: B=8, one batch element per NeuronCore, no
  collectives.
- Activations transposed [features, tokens]; residual stream in fp32r,
  everything on matmul paths in float16 (weights pre-tiled on host into
  contiguous per-output-tile blocks for max DMA efficiency).
- Softmax: exp(scores) on scalar engine (f16), multiplicative exp(bias)
  (host-precomputed, f16) on vector; the kv-sum (softmax denominator) is
  folded into the P@V matmul via V tiles laid out [64 ones | 64 values]
  per head: psum rows 0:64 = sums, 64:128 = P@V.
- DMA spread across sync/scalar/gpsimd rings; output written per-psum-bank
  with rotating buffers to avoid a serialized tail.
"""
import sys

sys.path.insert(0, '/opt/trn_rl_repo')

import numpy as np

E, C, H, D, FF = 768, 512, 12, 64, 3072
B, S, L = 8, 1024, 256
EPS = 1e-5
NCORES = 8
QCH = 512                  # q-chunk (matmul moving free dim)
NQ = S // QCH              # 2
JE = E // 128              # 6 feature tiles
JC = C // 128              # 4 cross-feature tiles
JF = FF // 128             # 24 ffn tiles
KVS = S // 128             # 8 self kv tiles
KVC = L // 128             # 2 cross kv tiles

_BUILT = {}
TRACE = False
LAST = {}
PHASES = []


def _build(flags):
    import concourse.bacc as bacc
    import concourse.mybir as mybir
    import concourse.tile as tile
    from concourse.tile import add_dep_helper
    from concourse.bass import AP as BassAP

    FR = mybir.dt.float32r
    F32 = mybir.dt.float32
    F16 = mybir.dt.float16
    AF = mybir.ActivationFunctionType
    OP = mybir.AluOpType

    nc = bacc.Bacc("TRN2", target_bir_lowering=False, debug=False,
                   enable_asserts=True, num_devices=NCORES)

    def din(name, shape, dt=F16):
        return nc.dram_tensor(name, shape, dt, kind="ExternalInput").ap()

    xT_d = din("xT", [E, S], F32)
    ctxT_d = din("ctxT", [C, L])
    # pre-tiled weights: [ofn, 128, jin*128] f16 (contiguous per of)
    w_d = {
        'wq_c': din("wq_c", [JE, 128, JE * 128]),
        'wk_c': din("wk_c", [JE, 128, JC * 128]),
        'wo_c': din("wo_c", [JE, 128, JE * 128]),
        'wq_s': din("wq_s", [JE, 128, JE * 128]),
        'wk_s': din("wk_s", [JE, 128, JE * 128]),
        'wo_s': din("wo_s", [JE, 128, JE * 128]),
        'w1': din("w1", [JF, 128, JE * 128]),
        'w2': din("w2", [JF, 128, JE * 128]),
        # V weights: [jin, 128, E] (row blocks of original)
        'wv_c': din("wv_c", [JC, 128, E]),
        'wv_s': din("wv_s", [JE, 128, E]),
    }
    expb_c_d = din("expb_c", [H, L, S])
    expb_s_d = din("expb_s", [H, S, S])
    VIDX = {}
    _off = 0
    for nm, ln in [('cn_g', JE), ('cn_b', JE), ('sn_g', JE), ('sn_b', JE),
                   ('fn_g', JE), ('fn_b', JE), ('bq_c', JE), ('bk_c', JE),
                   ('bo_c', JE), ('bq_s', JE), ('bk_s', JE), ('bo_s', JE),
                   ('b1', JF), ('b2', JE)]:
        VIDX[nm] = _off
        _off += ln
    NV = _off
    vecs_d = din("vecs", [128, NV], F32)
    yT_d = nc.dram_tensor("yT", [E, S], F32, kind="ExternalOutput").ap()

    with tile.TileContext(nc) as tc:
        with tc.tile_pool(name="const", bufs=1) as cpool, \
             tc.tile_pool(name="acts", bufs=1) as acts, \
             tc.tile_pool(name="wst", bufs=8) as wst, \
             tc.tile_pool(name="tr", bufs=2) as tr, \
             tc.tile_pool(name="pe", bufs=4) as pepool, \
             tc.tile_pool(name="eb", bufs=4) as ebpool, \
             tc.tile_pool(name="ps", bufs=1, space="PSUM") as ps:

            def T(pool, shape, dtype, tag, bufs=1):
                return pool.tile(shape, dtype, tag=tag, name=tag, bufs=bufs)

            ones = T(cpool, [128, 128], FR, "ones")
            ones16 = T(cpool, [128, 128], F16, "ones16")
            ones_f = T(cpool, [128, 128], F32, "ones_f")
            epsc = T(cpool, [128, 1], F32, "epsc")
            nc.vector.memset(epsc[:], EPS)
            nc.vector.memset(ones_f[:], 1.0)
            nc.vector.tensor_copy(ones[:], ones_f[:])
            nc.vector.tensor_copy(ones16[:], ones_f[:])
            vecs = T(cpool, [128, NV], F32, "vecs")
            nc.sync.dma_start(vecs[:], vecs_d[:])

            def vap(nm, j):
                return vecs[:, VIDX[nm] + j:VIDX[nm] + j + 1]

            # persistent activation tiles
            rA = [T(acts, [128, S], FR, f"rA{j}") for j in range(JE)]
            rB = [T(acts, [128, S], FR, f"rB{j}") for j in range(JE)]
            lnT = [T(acts, [128, S], F16, f"ln{j}") for j in range(JE)]
            KT = [T(acts, [128, S], F16, f"KT{j}") for j in range(JE)]
            # V tiles: per head 128 cols = [64 ones | 64 values]
            Vx = [T(acts, [128, H * 128], F16, f"V{t}") for t in range(KVS)]
            QT = [T(acts, [128, QCH], F16, f"QT{j}") for j in range(JE)]
            QTb = [T(acts, [128, QCH], F16, f"QU{j}") for j in range(JE)]
            AT = [T(acts, [128, QCH], F16, f"AT{j}") for j in range(JE)]
            ctxT = [T(acts, [128, L], F16, f"cx{j}") for j in range(JC)]

            def vx_strided(t, head0, nh, ones_cols):
                vp = Vx[t][:]
                pstride = vp.ap[0][0]
                off = vp.offset + head0 * 128 + (0 if ones_cols else 64)
                return BassAP(vp.tensor, off,
                              [[pstride, 128], [128, nh], [1, 64]])

            for t in range(KVS):
                nc.vector.memset(vx_strided(t, 0, H, True), 1.0)

            for j in range(JC):
                nc.sync.dma_start(ctxT[j][:], ctxT_d[j * 128:(j + 1) * 128, :])

            def load_xT():
                # emitted after cross-K/V weight DMAs: ring order = emission
                # order, so K/V weights land first and the PE starts at once
                for half in range(2):
                    cs_ = slice(half * QCH, (half + 1) * QCH)
                    for j in range(JE):
                        ring = nc.sync if j % 2 == 0 else nc.scalar
                        ring.dma_start(
                            rA[j][:, cs_],
                            xT_d[j * 128:(j + 1) * 128, cs_].bitcast(FR))

            # PSUM: 3x double-bank "scp" tiles (scores/general) + 2 single
            # "pv" banks = 8 banks total.
            def psum_tile(tag, n=QCH):
                return ps.tile([128, n], F32, tag=tag, name=tag)

            def scp_tile(k):
                return psum_tile(f"scp{k % 3}", 2 * QCH)

            _rot = {'i': 0, 'cur': None}

            def rot_ps(n=QCH):
                i = _rot['i']
                _rot['i'] += 1
                if i % 2 == 0:
                    _rot['cur'] = scp_tile(i // 2)
                return _rot['cur'][:, (i % 2) * QCH:(i % 2) * QCH + n]

            # ---------------- layer norm (transposed layout) ----------------
            def ln_phase(src, dst, gname, bname, affine, only_qc=None):
                inv = 1.0 / float(E)
                for qc in range(NQ):
                    if only_qc is not None and qc != only_qc:
                        continue
                    qs = slice(qc * QCH, (qc + 1) * QCH)
                    sqs = []
                    for j in range(JE):
                        sq = T(tr, [128, QCH], F16, "sq", bufs=6)
                        nc.scalar.activation(sq[:], src[j][:, qs], AF.Square)
                        sqs.append(sq)
                    s12 = scp_tile(0)
                    s1 = s12[:, 0:QCH]
                    s2 = s12[:, QCH:2 * QCH]
                    for j in range(JE):
                        nc.tensor.matmul(s1, ones[:, 0:128], src[j][:, qs],
                                         start=(j == 0), stop=(j == JE - 1))
                    for j in range(JE):
                        nc.tensor.matmul(s2, ones16[:, 0:128], sqs[j][:],
                                         start=(j == 0), stop=(j == JE - 1))
                    t1 = T(tr, [128, QCH], F32, "t1m", bufs=1)
                    nc.scalar.activation(t1[:], s1[:], AF.Square, scale=inv)
                    var = T(tr, [128, QCH], F32, "var", bufs=1)
                    nc.vector.scalar_tensor_tensor(var[:], s2[:], inv, t1[:],
                                                   op0=OP.mult, op1=OP.subtract)
                    nc.scalar.activation(var[:], var[:], AF.Sqrt,
                                         bias=epsc[:, 0:1])
                    rstd = T(tr, [128, QCH], F32, "rstd", bufs=1)
                    nc.vector.reciprocal_approx_fast(rstd[:], var[:])
                    m1r = T(tr, [128, QCH], F32, "m1r", bufs=1)
                    nc.vector.scalar_tensor_tensor(m1r[:], s1[:], inv, rstd[:],
                                                   op0=OP.mult, op1=OP.mult)
                    for j in range(JE):
                        tmp = T(tr, [128, QCH], F32, "lntmp", bufs=2)
                        nc.vector.tensor_tensor(tmp[:], src[j][:, qs], rstd[:],
                                                op=OP.mult)
                        if affine:
                            tmp2 = T(tr, [128, QCH], F32, "lntmp2", bufs=2)
                            nc.vector.tensor_tensor(tmp2[:], tmp[:], m1r[:],
                                                    op=OP.subtract)
                            nc.vector.tensor_scalar(dst[j][:, qs], tmp2[:],
                                                    vap(gname, j), vap(bname, j),
                                                    op0=OP.mult, op1=OP.add)
                        else:
                            nc.vector.tensor_tensor(dst[j][:, qs], tmp[:],
                                                    m1r[:], op=OP.subtract)

            # -------- projection from pre-tiled weights --------
            def wload(wd, of, jin, ring=None):
                wt = T(wst, [128, JE * 128], F16, "wg", bufs=6)
                (ring or nc.sync).dma_start(wt[:, 0:jin * 128], wd[of])
                return wt

            _cpn = {'i': 0}

            def wchain(wt, jin, src_getter, out_ap, n, bias_ap, pt=None):
                if pt is None:
                    pt = rot_ps(n)
                for j in range(jin):
                    nc.tensor.matmul(pt, wt[:, j * 128:(j + 1) * 128],
                                     src_getter(j),
                                     start=(j == 0), stop=(j == jin - 1))
                if bias_ap is not None:
                    nc.vector.tensor_scalar(out_ap, pt, bias_ap, None,
                                            op0=OP.add)
                elif _cpn['i'] % 2 == 0:
                    _cpn['i'] += 1
                    nc.scalar.copy(out_ap, pt)
                else:
                    _cpn['i'] += 1
                    nc.vector.tensor_copy(out_ap, pt)

            def qproj_closures(prefix, wq, lnt, qs, qt_set, pslot=None):
                outs = []
                for of in range(JE):
                    def one(of=of):
                        wt = wload(wq, of, JE)
                        wchain(wt, JE, lambda j: lnt[j][:, qs],
                               qt_set[of][:], QCH,
                               vap(f'bq_{prefix}', of)
                               if flags[f'bq_{prefix}'] else None,
                               pt=pslot() if pslot else None)
                    outs.append(one)
                return outs

            _frot = {'i': 0, 'cur': None}

            def fill_ps(n=QCH):
                i = _frot['i']
                _frot['i'] += 1
                if i % 2 == 0:
                    _frot['cur'] = psum_tile("scp2", 2 * QCH)
                return _frot['cur'][:, (i % 2) * QCH:(i % 2) * QCH + n]

            # ---------------- K/V projection emission ----------------
            def emit_kv(prefix, kv_src, wk, wv, jin_kv, kv_len):
                nkv = kv_len // 128

                _kps = {'i': 0}

                def emit_k(of):
                    wt = wload(wk, of, jin_kv)
                    for ks in range(0, kv_len, QCH):
                        n = min(QCH, kv_len - ks)
                        kp = psum_tile(f"pv{_kps['i'] % 2}")
                        _kps['i'] += 1
                        wchain(wt, jin_kv,
                               lambda j: kv_src[j][:, ks:ks + n],
                               KT[of][:, ks:ks + n], n,
                               vap(f'bk_{prefix}', of)
                               if flags[f'bk_{prefix}'] else None,
                               pt=kp[:, 0:n])

                def emit_vgroup(os_, tg):
                    n = min(QCH, E - os_)
                    tcnt = min(4, nkv - tg)
                    vts = [scp_tile(1), scp_tile(2)]
                    vps = [vts[i // 2][:, (i % 2) * QCH:(i % 2) * QCH + n]
                           for i in range(tcnt)]
                    for j in range(jin_kv):
                        wt = T(wst, [128, QCH], F16, "wv", bufs=3)
                        nc.sync.dma_start(wt[:, 0:n], wv[j, :, os_:os_ + n])
                        for i in range(tcnt):
                            nc.tensor.matmul(
                                vps[i][:, 0:n],
                                kv_src[j][:, (tg + i) * 128:(tg + i + 1) * 128],
                                wt[:, 0:n], start=(j == 0),
                                stop=(j == jin_kv - 1))
                    for i in range(tcnt):
                        dst = vx_strided(tg + i, os_ // 64, n // 64, False)
                        src = vps[i][:, 0:n].rearrange("p (h d) -> p h d", d=64)
                        nc.scalar.copy(dst, src)

                vgroups = [(os_, tg) for os_ in range(0, E, QCH)
                           for tg in range(0, nkv, 4)]
                for i in range(max(JE, len(vgroups))):
                    if i < JE:
                        emit_k(i)
                    if i < len(vgroups):
                        emit_vgroup(*vgroups[i])

            # ---------------- attention (shared cross/self) ----------------
            def attention(prefix, lnt, kv_src, expb_d, res_in,
                          res_out, wq, wk, wv, wo, jin_kv, kv_len,
                          post_qc=None, kv_done=False, qt_sets=None,
                          emit_q=(True, True), fillers=None, rot_mod=3):
                if not kv_done:
                    emit_kv(prefix, kv_src, wk, wv, jin_kv, kv_len)
                if qt_sets is None:
                    qt_sets = [QT, QT]

                for qc in range(NQ):
                    PHASES.append((f'{prefix}:qc{qc}',
                                   int(__import__('re').findall(
                                       r'\d+', nc.get_next_instruction_name())[0])))
                    qs = slice(qc * QCH, (qc + 1) * QCH)
                    qt = qt_sets[qc]
                    # Q^T for this q-chunk (scale folded into wq on host)
                    if emit_q[qc]:
                        for fn_ in qproj_closures(prefix, wq, lnt, qs, qt):
                            fn_()
                    fq = list(fillers[qc]) if fillers else []
                    nkt = kv_len // 128
                    npair = nkt // 2
                    seq = [(h, kp) for h in range(H) for kp in range(npair)]
                    _sr = {'i': 0}
                    state = {}

                    def load_eb(h):
                        ebts = []
                        for kp in range(npair):
                            ebt = T(ebpool, [128, 2 * QCH], F16, "eb", bufs=8)
                            ring = nc.gpsimd
                            ring.dma_start(
                                ebt[:].rearrange("p (t c) -> p t c", t=2),
                                expb_d[h, kp * 256:(kp + 1) * 256, qs]
                                .rearrange("(t p) c -> p t c", p=128))
                            ebts.append(ebt)
                        state.setdefault(h, {'tiles': []})['ebts'] = ebts

                    load_eb(0)

                    def s_stage(i):
                        h, kp = seq[i]
                        st = state.setdefault(h, {'tiles': []})
                        if kp == 0 and h + 1 < H:
                            load_eb(h + 1)
                        th, ph = (h * D) // 128, (h * D) % 128
                        sc = scp_tile(_sr['i'] % rot_mod)
                        _sr['i'] += 1
                        for half in range(2):
                            kvt = 2 * kp + half
                            chain(nc.tensor.matmul(
                                sc[:, half * QCH:(half + 1) * QCH],
                                KT[th][ph:ph + D, kvt * 128:(kvt + 1) * 128],
                                qt[th][ph:ph + D, :], start=True, stop=True))
                        pe = T(pepool, [128, 2 * QCH], F16, "pe", bufs=6)
                        nc.scalar.activation(pe[:], sc[:], AF.Exp)
                        nc.vector.tensor_tensor(pe[:], pe[:],
                                                st['ebts'][kp][:], op=OP.mult)
                        st['tiles'].append(pe)

                    def pv_stage(i):
                        h, kp = seq[i]
                        st = state[h]
                        th, ph = (h * D) // 128, (h * D) % 128
                        if kp == 0:
                            st['pv'] = psum_tile(f"pv{h % 2}")
                        pe = st['tiles'][kp]
                        for half in range(2):
                            kvt = 2 * kp + half
                            chain(nc.tensor.matmul(
                                st['pv'][:], Vx[kvt][:, h * 128:(h + 1) * 128],
                                pe[:, half * QCH:(half + 1) * QCH],
                                start=(kvt == 0), stop=(kvt == nkt - 1)))
                        if kp == npair - 1:
                            pv = st['pv']

                            def fin(pv=pv, th=th, ph=ph):
                                # psum rows 0:64 = kv-sums, 64:128 = P@V
                                rec = T(tr, [64, QCH], F32, "rec", bufs=2)
                                nc.vector.reciprocal_approx_fast(
                                    rec[:], pv[0:64, :])
                                nc.vector.tensor_tensor(AT[th][ph:ph + D, :],
                                                        pv[64:128, :],
                                                        rec[:], op=OP.mult)
                            deferred.append(fin)
                            del state[h]

                    _pe_chain = {'prev': None}

                    def chain(bi):
                        if _pe_chain['prev'] is not None:
                            add_dep_helper(bi.ins, _pe_chain['prev'].ins,
                                           sync=False, reason="pe-order")
                        _pe_chain['prev'] = bi

                    deferred = []
                    BLK = 3
                    blocks = [list(range(i, min(i + BLK, len(seq))))
                              for i in range(0, len(seq), BLK)]
                    for j in range(len(blocks) + 1):
                        if j < len(blocks):
                            for i in blocks[j]:
                                s_stage(i)
                        if fq and j >= 1:
                            fq.pop(0)()
                        while len(deferred) > 1:
                            deferred.pop(0)()
                        if j >= 1:
                            for i in blocks[j - 1]:
                                pv_stage(i)
                    while deferred:
                        deferred.pop(0)()
                    for fn_ in fq:
                        fn_()
                    # out-projection + residual
                    for of in range(JE):
                        wt = wload(wo, of, JE)
                        pt = rot_ps()
                        for j in range(JE):
                            nc.tensor.matmul(pt, wt[:, j * 128:(j + 1) * 128],
                                             AT[j][:],
                                             start=(j == 0), stop=(j == JE - 1))
                        if flags[f'bo_{prefix}']:
                            nc.vector.scalar_tensor_tensor(
                                res_out[of][:, qs], pt, vap(f'bo_{prefix}', of),
                                res_in[of][:, qs], op0=OP.add, op1=OP.add)
                        else:
                            nc.vector.tensor_tensor(res_out[of][:, qs], pt,
                                                    res_in[of][:, qs], op=OP.add)
                    if post_qc is not None:
                        post_qc(qc)

            # ================= the layer =================
            import re as _re

            def _mark(lbl):
                n = int(_re.findall(r'\d+', nc.get_next_instruction_name())[0])
                PHASES.append((lbl, n))

            _mark('ckv')
            # cross K/V only need ctxT: emit first to overlap xT DMA
            emit_kv('c', ctxT, w_d['wk_c'], w_d['wv_c'], JC, L)
            load_xT()
            _mark('ln1')
            ln_phase(rA, lnT, 'cn_g', 'cn_b', flags['cn'])
            _mark('cross')
            qs1 = slice(QCH, 2 * QCH)
            qs0 = slice(0, QCH)
            fill_q0 = qproj_closures('c', w_d['wq_c'], lnT, qs1, QTb,
                                     pslot=fill_ps)
            fill_q1 = qproj_closures('s', w_d['wq_s'], lnT, qs0, QT,
                                     pslot=fill_ps)
            attention('c', lnT, ctxT, expb_c_d, rA, rB,
                      w_d['wq_c'], w_d['wk_c'], w_d['wv_c'], w_d['wo_c'],
                      JC, L, kv_done=True, rot_mod=2,
                      qt_sets=[QT, QTb], emit_q=(True, False),
                      fillers=[fill_q0, fill_q1],
                      post_qc=lambda qc: ln_phase(rB, lnT, 'sn_g', 'sn_b',
                                                  flags['sn'], only_qc=qc))
            _mark('self')
            attention('s', lnT, lnT, expb_s_d, rB, rA,
                      w_d['wq_s'], w_d['wk_s'], w_d['wv_s'], w_d['wo_s'],
                      JE, S, qt_sets=[QT, QTb], emit_q=(False, True))
            # fn-ln after self-attn: overlaps FFN weight prefetch/start
            ln_phase(rA, lnT, 'fn_g', 'fn_b', flags['fn'])
            _mark('ffn')

            # ================= FFN =================
            for qc in range(NQ):
                qs = slice(qc * QCH, (qc + 1) * QCH)
                ytiles = [scp_tile(k) for k in range(3)]
                ypt = [ytiles[k // 2][:, (k % 2) * QCH:(k % 2 + 1) * QCH]
                       for k in range(JE)]

                def emit_f1(of):
                    wt = T(wst, [128, JE * 128], F16, "w1g", bufs=4)
                    nc.sync.dma_start(wt[:], w_d['w1'][of])
                    f1 = psum_tile(f"pv{of % 2}")
                    for j in range(JE):
                        nc.tensor.matmul(f1[:], wt[:, j * 128:(j + 1) * 128],
                                         lnT[j][:, qs],
                                         start=(j == 0), stop=(j == JE - 1))
                    g = T(tr, [128, QCH], F16, "gelu", bufs=3)
                    nc.scalar.activation(g[:], f1[:], AF.Gelu_apprx_tanh,
                                         bias=vap('b1', of) if flags['b1'] else 0.0)
                    return g

                def load_w2(of):
                    w2t = T(wst, [128, JE * 128], F16, "w2g", bufs=4)
                    nc.gpsimd.dma_start(w2t[:], w_d['w2'][of])
                    return w2t

                gprev = emit_f1(0)
                w2prev = load_w2(0)
                for of in range(JF):
                    gnext = emit_f1(of + 1) if of + 1 < JF else None
                    w2next = load_w2(of + 1) if of + 1 < JF else None
                    for of2 in range(JE):
                        nc.tensor.matmul(ypt[of2],
                                         w2prev[:, of2 * 128:(of2 + 1) * 128],
                                         gprev[:],
                                         start=(of == 0), stop=(of == JF - 1))
                    gprev = gnext
                    w2prev = w2next
                for of2 in range(JE):
                    yo = T(tr, [128, QCH], F32, "yout", bufs=6)
                    if flags['b2']:
                        nc.vector.tensor_scalar(yo[:], ypt[of2], vap('b2', of2),
                                                None, op0=OP.add)
                    else:
                        nc.vector.tensor_copy(yo[:], ypt[of2])
                    ring = [nc.sync, nc.scalar][of2 % 2]
                    ring.dma_start(yT_d[of2 * 128:(of2 + 1) * 128, qs], yo[:])

    nc.compile()
    return nc


def kernel(**inputs):
    inp = {k: np.asarray(v, dtype=np.float32) for k, v in inputs.items()}
    triv1 = lambda v: bool(np.all(v == 1.0))
    triv0 = lambda v: bool(np.all(v == 0.0))
    flags = {
        'cn': not (triv1(inp['cn_g']) and triv0(inp['cn_b'])),
        'sn': not (triv1(inp['sn_g']) and triv0(inp['sn_b'])),
        'fn': not (triv1(inp['fn_g']) and triv0(inp['fn_b'])),
        'bq_c': not triv0(inp['bq_c']), 'bk_c': not triv0(inp['bk_c']),
        'bo_c': not triv0(inp['bo_c']), 'bq_s': not triv0(inp['bq_s']),
        'bk_s': not triv0(inp['bk_s']), 'bo_s': not triv0(inp['bo_s']),
        'b1': not triv0(inp['b1']), 'b2': not triv0(inp['b2']),
    }
    assert triv0(inp['bv_c']) and triv0(inp['bv_s']), \
        "nonzero V bias not supported by this build"

    key = tuple(sorted(flags.items()))
    if key not in _BUILT:
        _BUILT[key] = _build(flags)
    nc = _BUILT[key]

    from concourse.bass_utils import run_bass_kernel_spmd

    f16 = np.float16
    scale = 1.0 / np.sqrt(np.float32(D))

    def tile_w(W, jin, ofn):
        return np.ascontiguousarray(
            W.reshape(jin, 128, ofn, 128).transpose(2, 1, 0, 3)
            .reshape(ofn, 128, jin * 128).astype(f16))

    def tile_v(W, jin):
        return np.ascontiguousarray(W.reshape(jin, 128, E).astype(f16))

    com = {
        'wq_c': tile_w(inp['wq_c'] * scale, JE, JE),
        'wk_c': tile_w(inp['wk_c'], JC, JE),
        'wv_c': tile_v(inp['wv_c'], JC),
        'wo_c': tile_w(inp['wo_c'], JE, JE),
        'wq_s': tile_w(inp['wq_s'] * scale, JE, JE),
        'wk_s': tile_w(inp['wk_s'], JE, JE),
        'wv_s': tile_v(inp['wv_s'], JE),
        'wo_s': tile_w(inp['wo_s'], JE, JE),
        'w1': tile_w(inp['w1'], JE, JF),
        'w2': np.ascontiguousarray(inp['w2'].reshape(JF, 128, E).astype(f16)),
        'expb_c': np.ascontiguousarray(
            np.exp(inp['bias_c'].transpose(0, 2, 1)).astype(f16)),
        'expb_s': np.ascontiguousarray(
            np.exp(inp['bias_s'].transpose(0, 2, 1)).astype(f16)),
    }
    chunks = []
    for nm in ['cn_g', 'cn_b', 'sn_g', 'sn_b', 'fn_g', 'fn_b']:
        chunks.append(inp[nm].reshape(-1, 128))
    chunks.append((inp['bq_c'] * scale).reshape(-1, 128))
    for nm in ['bk_c', 'bo_c']:
        chunks.append(inp[nm].reshape(-1, 128))
    chunks.append((inp['bq_s'] * scale).reshape(-1, 128))
    for nm in ['bk_s', 'bo_s', 'b1', 'b2']:
        chunks.append(inp[nm].reshape(-1, 128))
    com['vecs'] = np.ascontiguousarray(np.concatenate(chunks, 0).T)

    in_maps = []
    for b in range(B):
        m = dict(com)
        m['xT'] = np.ascontiguousarray(inp['hidden_state'][b].T)
        m['ctxT'] = np.ascontiguousarray(inp['context'][b].T.astype(f16))
        in_maps.append(m)

    res = run_bass_kernel_spmd(nc, in_maps, core_ids=list(range(NCORES)),
                               trace=TRACE)
    LAST['res'] = res
    y = np.stack([res.results[c]['yT'].T for c in range(B)])
    return np.ascontiguousarray(y.astype(np.float32))
